# revision 1
# baseline (speedup 1.0000x reference)
"""DIFFormerConv (simple linear attention + dense GCN) on 8 trn2 NeuronCores.

Sharding: nodes N=4096 split 8 ways (S=512 per core). Each core computes
q/k/v for its node shard, partial kvs/ks_sum/vsum (AllReduce), vmean
(AllGather, bf16), the attention output rows for its shard, and the GCN
rows for its shard (adj^T column shard, bf16 matmul).

Layouts chosen so no PE transposes are needed:
  q:   [hd, s]  (heads*dim on partitions)   -- lhsT = W^T chunks
  k,v: [s, hd]  (transposed projection)     -- lhsT = x chunks
  gcn: [(b,d), n] directly                  -- lhsT = vmean[m,(b,d)], rhs = adjT[m,n]
Host prep: adjT = adj.T + I (bf16), rrs = 0.25/(rowsum+1), W transposes.
"""

import sys

sys.path.insert(0, "/opt/trn_rl_repo")

import numpy as np
import ml_dtypes

from concourse import bass, bacc, tile, mybir
from concourse.bass_utils import run_bass_kernel_spmd

B, C, N, H, D = 8, 256, 4096, 4, 64
NCORES = 8
S = N // NCORES          # 512 nodes per core
HD = H * D               # 256
F32 = mybir.dt.float32
F32R = mybir.dt.float32r
BF16 = mybir.dt.bfloat16
AX = mybir.AxisListType.X
ALU = mybir.AluOpType
ACTF = mybir.ActivationFunctionType
RG = [list(range(NCORES))]

_CACHE = {}
DEBUG_DUMPS = False


def _indicators():
    i4a = np.zeros((128, 4), np.float32)
    i4b = np.zeros((128, 4), np.float32)
    for p in range(128):
        i4a[p, p // 64] = 1.0
        i4b[p, 2 + p // 64] = 1.0
    ibc0 = np.zeros((4, 128), np.float32)
    ibc1 = np.zeros((4, 128), np.float32)
    for p in range(128):
        ibc0[p // 64, p] = 1.0
        ibc1[2 + p // 64, p] = 1.0
    return i4a, i4b, ibc0, ibc1


def _build():
    nc = bacc.Bacc("TRN2", target_bir_lowering=False, debug=False,
                   num_devices=NCORES)

    xq = nc.dram_tensor("xq", [B, 2, 128, S], F32R, kind="ExternalInput")
    xs = nc.dram_tensor("xs", [B, 2, 128, S], F32R, kind="ExternalInput")
    adjt = nc.dram_tensor("adjt", [32, 128, S], BF16, kind="ExternalInput")
    rrs = nc.dram_tensor("rrs", [1, S], F32R, kind="ExternalInput")
    wqt = nc.dram_tensor("wqt", [2, 128, HD], F32R, kind="ExternalInput")
    wkt = nc.dram_tensor("wkt", [2, 128, HD], F32R, kind="ExternalInput")
    wvt = nc.dram_tensor("wvt", [2, 128, HD], F32R, kind="ExternalInput")
    bqr = nc.dram_tensor("bqr", [1, HD], F32R, kind="ExternalInput")
    bkr = nc.dram_tensor("bkr", [1, HD], F32R, kind="ExternalInput")
    bvr = nc.dram_tensor("bvr", [1, HD], F32R, kind="ExternalInput")
    out = nc.dram_tensor("out", [B, D, S], F32, kind="ExternalOutput")
    if DEBUG_DUMPS:
        dbg_ar = nc.dram_tensor("dbg_ar", [B, 2, 132, D], F32,
                                kind="ExternalOutput")
        dbg_vm = nc.dram_tensor("dbg_vm", [NCORES, S, B, D], BF16,
                                kind="ExternalOutput")

    i4a_d = nc.dram_tensor("i4a_in", [128, 4], F32R, kind="ExternalInput")
    i4b_d = nc.dram_tensor("i4b_in", [128, 4], F32R, kind="ExternalInput")
    ibc0_d = nc.dram_tensor("ibc0_in", [4, 128], F32R, kind="ExternalInput")
    ibc1_d = nc.dram_tensor("ibc1_in", [4, 128], F32R, kind="ExternalInput")
    ones_r_d = nc.dram_tensor("ones_r", [1, S], F32R, kind="ExternalInput")
    ones_c_d = nc.dram_tensor("ones_c", [128, 1], F32R, kind="ExternalInput")

    def r(ap):
        return ap

    with nc.allow_low_precision(reason="float32r rounding intentional"), \
            tile.TileContext(nc) as tc:
        with (
            tc.tile_pool(name="pers", bufs=1) as pp,
            tc.tile_pool(name="work", bufs=3) as wk,
            tc.tile_pool(name="dram", bufs=1, space="DRAM") as dp,
        ):
            # DRAM internal buffers for collectives
            vm_loc = dp.tile([S, B, D], BF16, tag="vm_loc", name="vm_loc")
            vm_all = dp.tile([NCORES, S, B, D], BF16, tag="vm_all", name="vm_all", addr_space="Shared")
            ar_in = dp.tile([B, 2, 132, D], F32, tag="ar_in", name="ar_in")
            ar_out = dp.tile([B, 2, 132, D], F32, tag="ar_out", name="ar_out", addr_space="Shared")

            # ---- constants ----
            wq_t = [pp.tile([128, HD], F32R, tag=f"wq{c}", name=f"wq{c}") for c in range(2)]
            wk_t = [pp.tile([128, HD], F32R, tag=f"wk{c}", name=f"wk{c}") for c in range(2)]
            wv_t = [pp.tile([128, HD], F32R, tag=f"wv{c}", name=f"wv{c}") for c in range(2)]
            for c in range(2):
                nc.sync.dma_start(out=wq_t[c][:], in_=wqt[c])
                nc.sync.dma_start(out=wk_t[c][:], in_=wkt[c])
                nc.sync.dma_start(out=wv_t[c][:], in_=wvt[c])
            bq_row = pp.tile([1, HD], F32R, tag="bqrow")
            bk_row = pp.tile([1, HD], F32R, tag="bkrow")
            bv_row = pp.tile([1, HD], F32R, tag="bvrow")
            nc.sync.dma_start(out=bq_row[:], in_=bqr[:])
            nc.sync.dma_start(out=bk_row[:], in_=bkr[:])
            nc.sync.dma_start(out=bv_row[:], in_=bvr[:])
            i4a = pp.tile([128, 4], F32R, tag="i4a")
            i4b = pp.tile([128, 4], F32R, tag="i4b")
            ibc0 = pp.tile([4, 128], F32R, tag="ibc0")
            ibc1 = pp.tile([4, 128], F32R, tag="ibc1")
            nc.sync.dma_start(out=i4a[:], in_=i4a_d[:])
            nc.sync.dma_start(out=i4b[:], in_=i4b_d[:])
            nc.sync.dma_start(out=ibc0[:], in_=ibc0_d[:])
            nc.sync.dma_start(out=ibc1[:], in_=ibc1_d[:])
            ones_row = pp.tile([1, S], F32R, tag="ones_row")
            ones_col = pp.tile([128, 1], F32R, tag="ones_col")
            nc.sync.dma_start(out=ones_row[:], in_=ones_r_d[:])
            nc.sync.dma_start(out=ones_col[:], in_=ones_c_d[:])
            rrs_row = pp.tile([1, S], F32R, tag="rrs_row")
            nc.sync.dma_start(out=rrs_row[:], in_=rrs[:])

            # persistent per-batch SBUF tensors
            q_sb = [[pp.tile([128, S], F32R, tag=f"q{b}_{h}", name=f"q{b}_{h}") for h in range(2)]
                    for b in range(B)]
            kt_sb = [[pp.tile([128, HD], F32R, tag=f"kt{b}_{s}", name=f"kt{b}_{s}") for s in range(4)]
                     for b in range(B)]
            vt_sb = [[pp.tile([128, HD], F32R, tag=f"vt{b}_{s}", name=f"vt{b}_{s}") for s in range(4)]
                     for b in range(B)]
            rq_sb = [pp.tile([4, S], F32, tag=f"rq{b}", name=f"rq{b}") for b in range(B)]
            attn_sb = [pp.tile([128, S], F32, tag=f"at{p}", name=f"at{p}")
                       for p in range(4)]
            rrs_bc = pp.tile([128, S], F32, tag="rrs_bc")

            with tc.tile_pool(name="psA", bufs=1, space="PSUM") as psA:
                # broadcast rrs row to all 128 partitions (K=1 matmul)
                pbc0 = psA.tile([128, S], F32, tag="pq")
                nc.tensor.matmul(pbc0[:], lhsT=r(ones_row[:, 0:128]),
                                 rhs=r(rrs_row[:]), start=True, stop=True)
                nc.scalar.activation(rrs_bc[:], pbc0[:], ACTF.Copy)

                # =================== phase 1: per-batch local ===================
                for b in range(B):
                    xs0 = wk.tile([128, S], F32R, tag="xs0", bufs=2)
                    xs1 = wk.tile([128, S], F32R, tag="xs1", bufs=2)
                    nc.sync.dma_start(out=xs0[:], in_=xs[b, 0])
                    nc.sync.dma_start(out=xs1[:], in_=xs[b, 1])

                    kvs_ps0 = psA.tile([128, HD], F32, tag="kvs0")
                    kvs_ps1 = psA.tile([128, HD], F32, tag="kvs1")
                    ks_ps = psA.tile([1, HD], F32, tag="ksps")
                    vs_ps = psA.tile([1, HD], F32, tag="vsps")

                    for sb_i in range(4):
                        sl = slice(sb_i * 128, (sb_i + 1) * 128)
                        # k^T and v^T projections: out[s, hd]
                        pk = psA.tile([128, HD], F32, tag="pk")
                        pv = psA.tile([128, HD], F32, tag="pv")
                        for (ps, wt, brow) in ((pk, wk_t, bk_row),
                                               (pv, wv_t, bv_row)):
                            nc.tensor.matmul(ps[:], lhsT=r(xs0[:, sl]),
                                             rhs=r(wt[0][:]), start=True,
                                             stop=False)
                            nc.tensor.matmul(ps[:], lhsT=r(xs1[:, sl]),
                                             rhs=r(wt[1][:]), start=False,
                                             stop=False)
                            nc.tensor.matmul(ps[:], lhsT=r(ones_row[:, 0:128]),
                                             rhs=r(brow[:]), start=False,
                                             stop=True)
                        # v^T evac
                        nc.scalar.activation(vt_sb[b][sb_i][:], pv[:], ACTF.Copy)
                        # vmean (sum over heads; /4 folded into rrs) -> bf16
                        vm_t = wk.tile([128, D], BF16, tag="vmt")
                        with nc.allow_low_precision(reason="vmean bf16 is ok"):
                            nc.vector.reduce_sum(
                                vm_t[:], pv[:].rearrange("p (h d) -> p d h",
                                                         h=H),
                                axis=AX)
                        nc.sync.dma_start(out=vm_loc[sl, b, :], in_=vm_t[:])
                        # kn = k / ||k||  (per head, free-dim blocks of 64)
                        sq = wk.tile([128, HD], F32, tag="sq")
                        nc.scalar.activation(sq[:], pk[:], ACTF.Square)
                        ssk = wk.tile([128, H], F32, tag="ssk")
                        nc.vector.reduce_sum(
                            ssk[:], sq[:].rearrange("p (h d) -> p h d", h=H),
                            axis=AX)
                        snk = wk.tile([128, H], F32, tag="snk")
                        nc.scalar.activation(snk[:], ssk[:], ACTF.Sqrt)
                        rk = wk.tile([128, H], F32, tag="rk")
                        nc.vector.reciprocal(rk[:], snk[:])
                        for h in range(H):
                            dsl = slice(h * D, (h + 1) * D)
                            nc.vector.tensor_scalar_mul(
                                kt_sb[b][sb_i][:, dsl], pk[:, dsl],
                                rk[:, h:h + 1])

                    # kvs / ks_sum / vsum partials, one contiguous
                    # accumulation group per bank
                    for sb_i in range(4):
                        nc.tensor.matmul(kvs_ps0[:],
                                         lhsT=r(kt_sb[b][sb_i][:, 0:128]),
                                         rhs=r(vt_sb[b][sb_i][:]),
                                         start=(sb_i == 0), stop=(sb_i == 3))
                    for sb_i in range(4):
                        nc.tensor.matmul(kvs_ps1[:],
                                         lhsT=r(kt_sb[b][sb_i][:, 128:HD]),
                                         rhs=r(vt_sb[b][sb_i][:]),
                                         start=(sb_i == 0), stop=(sb_i == 3))
                    for sb_i in range(4):
                        nc.tensor.matmul(ks_ps[:], lhsT=r(ones_col[:]),
                                         rhs=r(kt_sb[b][sb_i][:]),
                                         start=(sb_i == 0), stop=(sb_i == 3))
                    for sb_i in range(4):
                        nc.tensor.matmul(vs_ps[:], lhsT=r(ones_col[:]),
                                         rhs=r(vt_sb[b][sb_i][:]),
                                         start=(sb_i == 0), stop=(sb_i == 3))

                    # evac kvs diag blocks (stacked [128,(h,m) x 64 d])
                    pk0 = wk.tile([128, D], F32, tag="arpk0")
                    pk1 = wk.tile([128, D], F32, tag="arpk1")
                    nc.scalar.activation(pk0[0:64, :], kvs_ps0[0:64, 0:64],
                                         ACTF.Copy)
                    nc.scalar.activation(pk0[64:128, :],
                                         kvs_ps0[64:128, 64:128], ACTF.Copy)
                    nc.scalar.activation(pk1[0:64, :], kvs_ps1[0:64, 128:192],
                                         ACTF.Copy)
                    nc.scalar.activation(pk1[64:128, :],
                                         kvs_ps1[64:128, 192:256], ACTF.Copy)
                    ksvs_sb = wk.tile([1, 2 * HD], F32, tag="ksvs_sb", bufs=2)
                    nc.scalar.activation(ksvs_sb[0:1, 0:HD], ks_ps[:],
                                         ACTF.Copy)
                    nc.scalar.activation(ksvs_sb[0:1, HD:2 * HD], vs_ps[:],
                                         ACTF.Copy)
                    nc.sync.dma_start(out=ar_in[b, 0, 0:128, :], in_=pk0[:])
                    nc.sync.dma_start(out=ar_in[b, 1, 0:128, :], in_=pk1[:])
                    nc.sync.dma_start(out=ar_in[b, 0, 128:130, :],
                                      in_=ksvs_sb[0:1, 0:128])
                    nc.sync.dma_start(out=ar_in[b, 1, 128:130, :],
                                      in_=ksvs_sb[0:1, 128:256])
                    nc.sync.dma_start(out=ar_in[b, 0, 130:132, :],
                                      in_=ksvs_sb[0:1, 256:384])
                    nc.sync.dma_start(out=ar_in[b, 1, 130:132, :],
                                      in_=ksvs_sb[0:1, 384:512])

                    # q projection: out[hd, s]
                    xq0 = wk.tile([128, S], F32R, tag="xs0", bufs=2)
                    xq1 = wk.tile([128, S], F32R, tag="xs1", bufs=2)
                    nc.sync.dma_start(out=xq0[:], in_=xq[b, 0])
                    nc.sync.dma_start(out=xq1[:], in_=xq[b, 1])
                    ss_ps = psA.tile([4, S], F32, tag="ss")
                    for h in range(2):
                        hsl = slice(h * 128, (h + 1) * 128)
                        pq = psA.tile([128, S], F32, tag="pq")
                        nc.tensor.matmul(pq[:], lhsT=r(wq_t[0][:, hsl]),
                                         rhs=r(xq0[:]), start=True, stop=False)
                        nc.tensor.matmul(pq[:], lhsT=r(wq_t[1][:, hsl]),
                                         rhs=r(xq1[:]), start=False, stop=False)
                        nc.tensor.matmul(pq[:], lhsT=r(bq_row[:, hsl]),
                                         rhs=r(ones_row[:]), start=False,
                                         stop=True)
                        nc.scalar.activation(q_sb[b][h][:], pq[:], ACTF.Copy)
                        qsq = wk.tile([128, S], F32R, tag="qsq", bufs=2)
                        nc.scalar.activation(qsq[:], pq[:], ACTF.Square)
                        nc.tensor.matmul(ss_ps[:],
                                         lhsT=r(i4a[:] if h == 0 else i4b[:]),
                                         rhs=r(qsq[:]), start=(h == 0),
                                         stop=(h == 1))
                    snq = wk.tile([4, S], F32, tag="snq", bufs=1)
                    nc.scalar.activation(snq[:], ss_ps[:], ACTF.Sqrt)
                    nc.vector.reciprocal(rq_sb[b][:], snq[:])

            # =================== collectives ===================
            nc.gpsimd.collective_compute(
                "AllGather", ALU.bypass, ins=[vm_loc.opt()],
                outs=[vm_all.opt()], replica_groups=RG)
            nc.gpsimd.collective_compute(
                "AllReduce", ALU.add, ins=[ar_in.opt()],
                outs=[ar_out.opt()], replica_groups=RG)

            # =================== phase 2: attention epilogue ===================
            with tc.tile_pool(name="psB", bufs=2, space="PSUM") as psB:
                for b in range(B):
                    kpk0f = wk.tile([128, D], F32, tag="kpk0f")
                    kpk1f = wk.tile([128, D], F32, tag="kpk1f")
                    nc.sync.dma_start(out=kpk0f[:], in_=ar_out[b, 0, 0:128, :])
                    nc.sync.dma_start(out=kpk1f[:], in_=ar_out[b, 1, 0:128, :])
                    kpk0 = wk.tile([128, D], F32R, tag="kpk0")
                    kpk1 = wk.tile([128, D], F32R, tag="kpk1")
                    nc.scalar.activation(kpk0[:], kpk0f[:], ACTF.Copy)
                    nc.scalar.activation(kpk1[:], kpk1f[:], ACTF.Copy)
                    ksp0f = wk.tile([128, 4], F32, tag="ksp0f")
                    ksp1f = wk.tile([128, 4], F32, tag="ksp1f")
                    nc.vector.memset(ksp0f[:], 0.0)
                    nc.vector.memset(ksp1f[:], 0.0)
                    nc.sync.dma_start(out=ksp0f[0:64, 0:1],
                                      in_=ar_out[b, 0, 128, :])
                    nc.sync.dma_start(out=ksp0f[64:128, 1:2],
                                      in_=ar_out[b, 0, 129, :])
                    nc.sync.dma_start(out=ksp1f[0:64, 2:3],
                                      in_=ar_out[b, 1, 128, :])
                    nc.sync.dma_start(out=ksp1f[64:128, 3:4],
                                      in_=ar_out[b, 1, 129, :])
                    ksp0 = wk.tile([128, 4], F32R, tag="ksp0")
                    ksp1 = wk.tile([128, 4], F32R, tag="ksp1")
                    nc.scalar.activation(ksp0[:], ksp0f[:], ACTF.Copy)
                    nc.scalar.activation(ksp1[:], ksp1f[:], ACTF.Copy)
                    vspf = wk.tile([4, D], F32, tag="vspf")
                    nc.sync.dma_start(out=vspf[0:2, :],
                                      in_=ar_out[b, 0, 130:132, :])
                    nc.sync.dma_start(out=vspf[2:4, :],
                                      in_=ar_out[b, 1, 130:132, :])
                    vsp = wk.tile([4, D], F32R, tag="vsp")
                    nc.scalar.activation(vsp[:], vspf[:], ACTF.Copy)

                    pden = psB.tile([4, S], F32, tag="pb")
                    nc.tensor.matmul(pden[:], lhsT=r(ksp0[:]),
                                     rhs=r(q_sb[b][0][:]), start=True,
                                     stop=False)
                    nc.tensor.matmul(pden[:], lhsT=r(ksp1[:]),
                                     rhs=r(q_sb[b][1][:]), start=False,
                                     stop=True)
                    t0 = wk.tile([4, S], F32, tag="t0", bufs=1)
                    nc.vector.tensor_mul(t0[:], pden[:], rq_sb[b][:])
                    t1 = wk.tile([4, S], F32, tag="t1", bufs=1)
                    nc.vector.tensor_scalar(t1[:], t0[:], 4.0, float(4 * N),
                                            op0=ALU.mult, op1=ALU.add)
                    rp = wk.tile([4, S], F32R, tag="rp", bufs=2)
                    nc.vector.reciprocal(rp[:], t1[:])  # 0.25/denom
                    cc = wk.tile([4, S], F32R, tag="cc", bufs=2)
                    nc.vector.tensor_mul(cc[:], rp[:].bitcast(F32), rq_sb[b][:])

                    pat = psB.tile([D, S], F32, tag="pat")
                    for h in range(2):
                        pbc = psB.tile([128, S], F32, tag="pb")
                        nc.tensor.matmul(pbc[:],
                                         lhsT=r(ibc0[:] if h == 0 else ibc1[:]),
                                         rhs=r(cc[:]), start=True, stop=True)
                        qs = wk.tile([128, S], F32R, tag="qs", bufs=2)
                        nc.vector.tensor_mul(qs[:], q_sb[b][h][:].bitcast(F32), pbc[:])
                        nc.tensor.matmul(pat[:],
                                         lhsT=r(kpk0[:] if h == 0 else kpk1[:]),
                                         rhs=r(qs[:]), start=(h == 0),
                                         stop=False)
                    nc.tensor.matmul(pat[:], lhsT=r(vsp[:]), rhs=r(rp[:]),
                                     start=False, stop=True)
                    nc.scalar.activation(
                        attn_sb[b // 2][(b % 2) * D:(b % 2 + 1) * D, :],
                        pat[:], ACTF.Copy)

                # =================== phase 3: GCN ===================
                with tc.tile_pool(name="psC", bufs=1, space="PSUM") as psC:
                    pg = [psC.tile([128, S], F32, tag=f"g{p}", name=f"g{p}") for p in range(4)]
                    for mc in range(32):
                        adj_t = wk.tile([128, S], BF16, tag="adj")
                        nc.sync.dma_start(out=adj_t[:], in_=adjt[mc])
                        for p in range(4):
                            vm_t = wk.tile([128, 128], BF16, tag="vml")
                            lc = mc % 4
                            nc.sync.dma_start(
                                out=vm_t[:],
                                in_=vm_all[mc // 4,
                                           lc * 128:(lc + 1) * 128,
                                           2 * p:2 * p + 2, :])
                            nc.tensor.matmul(pg[p][:], lhsT=vm_t[:],
                                             rhs=adj_t[:], start=(mc == 0),
                                             stop=(mc == 31))
                    for p in range(4):
                        gt = wk.tile([128, S], F32, tag="gt", bufs=2)
                        nc.vector.tensor_mul(gt[:], pg[p][:], rrs_bc[:])
                        ot = wk.tile([128, S], F32, tag="ot", bufs=2)
                        nc.vector.tensor_add(ot[:], gt[:], attn_sb[p][:])
                        nc.sync.dma_start(out=out[2 * p], in_=ot[0:D, :])
                        nc.sync.dma_start(out=out[2 * p + 1], in_=ot[D:128, :])
                    if DEBUG_DUMPS:
                        nc.sync.dma_start(out=dbg_ar[:], in_=ar_out[:])
                        nc.sync.dma_start(out=dbg_vm[:], in_=vm_all[:])
    nc.compile()
    return nc


def _prep_inputs(query_input, source_input, adj, Wq_w, Wq_b, Wk_w, Wk_b,
                 Wv_w, Wv_b):
    xq_np = np.asarray(query_input, dtype=np.float32)
    xs_np = np.asarray(source_input, dtype=np.float32)
    adj_np = np.asarray(adj, dtype=np.float32)

    adjT = np.ascontiguousarray(adj_np.T)
    np.fill_diagonal(adjT, adjT.diagonal() + 1.0)
    adjT_bf = adjT.astype(ml_dtypes.bfloat16)
    rrs_full = (0.25 / (adj_np.sum(axis=1) + 1.0)).astype(np.float32)

    wqt = np.ascontiguousarray(np.asarray(Wq_w, np.float32).T).reshape(2, 128, HD)
    wkt = np.ascontiguousarray(np.asarray(Wk_w, np.float32).T).reshape(2, 128, HD)
    wvt = np.ascontiguousarray(np.asarray(Wv_w, np.float32).T).reshape(2, 128, HD)
    bq = np.asarray(Wq_b, np.float32).reshape(1, HD)
    bk = np.asarray(Wk_b, np.float32).reshape(1, HD)
    bv = np.asarray(Wv_b, np.float32).reshape(1, HD)

    i4a, i4b, ibc0, ibc1 = _indicators()
    in_maps = []
    for i in range(NCORES):
        sl = slice(i * S, (i + 1) * S)
        in_maps.append({
            "xq": np.ascontiguousarray(xq_np[:, :, sl]).reshape(B, 2, 128, S),
            "xs": np.ascontiguousarray(xs_np[:, :, sl]).reshape(B, 2, 128, S),
            "adjt": np.ascontiguousarray(adjT_bf[:, sl]).reshape(32, 128, S),
            "rrs": np.ascontiguousarray(rrs_full[sl]).reshape(1, S),
            "wqt": wqt, "wkt": wkt, "wvt": wvt,
            "bqr": bq, "bkr": bk, "bvr": bv,
            "i4a_in": i4a, "i4b_in": i4b,
            "ibc0_in": ibc0, "ibc1_in": ibc1,
            "ones_r": np.ones((1, S), np.float32),
            "ones_c": np.ones((128, 1), np.float32),
        })
    return in_maps


def kernel(**inputs):
    if "nc" not in _CACHE:
        _CACHE["nc"] = _build()
    nc = _CACHE["nc"]
    in_maps = _prep_inputs(**inputs)
    res = run_bass_kernel_spmd(nc, in_maps, list(range(NCORES)))
    full = np.empty((B, D, N), np.float32)
    for i in range(NCORES):
        full[:, :, i * S:(i + 1) * S] = res.results[i]["out"]
    return full



# revision 25
# speedup vs baseline: 1.8265x; 1.8265x over previous
"""DIFFormerConv (simple linear attention + dense GCN) on 8 trn2 NeuronCores.

Sharding: nodes N=4096 split 8 ways (S=512 per core). Phase order is chosen
so both collectives hide under compute:
  V-pass (vmean -> fp8)  -> 4 chunked AllGathers start ~15% in
  K-pass + kvs partials  -> bf16 AllReduce (kvs diag blocks | ks cols | vs)
  Q-pass (weight-stationary, bf16)
  GCN (DoubleRow fp8: adjT resident in SBUF, vmean pairs as lhsT)
  attention epilogue (needs AllReduce, which landed during GCN)
  combine + store

Layouts (no PE transposes anywhere):
  q:   [hd, s]   (heads*dim on partitions)  -- lhsT = Wq^T chunks (stationary)
  k,v: [s, hd]   (transposed projection)    -- lhsT = xs chunks (stationary)
  kvs: lhsT = kt chunk, rhs = [vt | ones]   -- ks falls out as PSUM column 256
  gcn: [(b,d), n] -- lhsT = vm pairs [128,2,128] fp8, rhs = adjT [128,2,512]
Denominator algebra (one stacked [32,S] approx reciprocal):
  w1 = 1/(4*t + 4*N*sqrt(ss)),  w2 = 4*N*sqrt(ss)*w1,  t = q . ks (raw q)
  attn = sum_h kvs_h^T @ (q_h * w1_h) + (vs/(4N))^T @ w2
The PE on this part runs at 1.2 GHz regardless of HAM, so MM cycles are
minimized (DoubleRow fp8 GCN, N>=256 everywhere, dense issue order).
"""

import sys

sys.path.insert(0, "/opt/trn_rl_repo")

import numpy as np
import ml_dtypes

from concourse import bass, bacc, tile, mybir
from concourse.bass_utils import run_bass_kernel_spmd

B, C, N, H, D = 8, 256, 4096, 4, 64
NCORES = 8
S = N // NCORES          # 512 nodes per core
HD = H * D               # 256
F32 = mybir.dt.float32
BF16 = mybir.dt.bfloat16
FP8 = mybir.dt.float8e4
AX = mybir.AxisListType.X
ALU = mybir.AluOpType
ACTF = mybir.ActivationFunctionType
PERF = mybir.MatmulPerfMode
RG = [list(range(NCORES))]

USE_DOUBLE_ROW = True

_CACHE = {}


def _indicators():
    i4a = np.zeros((128, 4), np.float32)
    i4b = np.zeros((128, 4), np.float32)
    for p in range(128):
        i4a[p, p // 64] = 1.0
        i4b[p, 2 + p // 64] = 1.0
    ibcb = np.zeros((B, 2, 32, 128), np.float32)
    for b in range(B):
        for i in range(2):
            for p in range(128):
                ibcb[b, i, 4 * b + 2 * i + p // 64, p] = 1.0
    return i4a, i4b, ibcb


def _build():
    nc = bacc.Bacc("TRN2", target_bir_lowering=False, debug=False,
                   num_devices=NCORES)

    xq = nc.dram_tensor("xq", [B, 2, 128, S], BF16, kind="ExternalInput")
    xs = nc.dram_tensor("xs", [B, 2, 128, S], BF16, kind="ExternalInput")
    # DoubleRow layout: [peer*2+jp, ki, o, n] = adjT[peer*512+jp*256+o*128+ki, n]
    adjt = nc.dram_tensor("adjt", [16, 128, 2 * S], FP8, kind="ExternalInput")
    rrs = nc.dram_tensor("rrs", [1, S], BF16, kind="ExternalInput")
    wvt = nc.dram_tensor("wvt", [2, 128, HD], BF16, kind="ExternalInput")
    wkt = nc.dram_tensor("wkt", [2, 128, HD], BF16, kind="ExternalInput")
    wqt = nc.dram_tensor("wqt", [2, 128, HD], BF16, kind="ExternalInput")
    bvr = nc.dram_tensor("bvr", [1, HD], BF16, kind="ExternalInput")
    bkr = nc.dram_tensor("bkr", [1, HD], BF16, kind="ExternalInput")
    bqc = nc.dram_tensor("bqc", [2, 128, 1], F32, kind="ExternalInput")
    i4a_d = nc.dram_tensor("i4a_in", [128, 4], BF16, kind="ExternalInput")
    i4b_d = nc.dram_tensor("i4b_in", [128, 4], BF16, kind="ExternalInput")
    ibcb_d = nc.dram_tensor("ibcb_in", [B, 2, 32, 128], BF16,
                            kind="ExternalInput")
    ones_r_d = nc.dram_tensor("ones_r", [1, S], BF16, kind="ExternalInput")
    ones_c_d = nc.dram_tensor("ones_c", [128, 1], BF16, kind="ExternalInput")
    out = nc.dram_tensor("out", [B, D, S], F32, kind="ExternalOutput")

    with nc.allow_low_precision(reason="bf16/fp8 rounding intentional"), \
            tile.TileContext(nc) as tc:
        with (
            tc.tile_pool(name="pers", bufs=1) as pp,
            tc.tile_pool(name="work", bufs=3) as wk,
            tc.tile_pool(name="dram", bufs=1, space="DRAM") as dp,
        ):
            vm_loc = [dp.tile([128, B * D], FP8, tag=f"vml{j}",
                              name=f"vml{j}") for j in range(4)]
            vm_all = [dp.tile([NCORES, 128, B * D], FP8, tag=f"vma{j}",
                              name=f"vma{j}", addr_space="Shared")
                      for j in range(4)]
            ar_in = dp.tile([B, 2, 130, 66], BF16, tag="ar_in", name="ar_in")
            ar_out = dp.tile([B, 2, 130, 66], BF16, tag="ar_out",
                             name="ar_out", addr_space="Shared")

            # ---- constants ----
            wv_t = [pp.tile([128, HD], BF16, tag=f"wv{c}", name=f"wv{c}")
                    for c in range(2)]
            wk_t = [pp.tile([128, HD], BF16, tag=f"wk{c}", name=f"wk{c}")
                    for c in range(2)]
            wq_t = [pp.tile([128, HD], BF16, tag=f"wq{c}", name=f"wq{c}")
                    for c in range(2)]
            for c in range(2):
                nc.sync.dma_start(out=wv_t[c][:], in_=wvt[c])
                nc.sync.dma_start(out=wk_t[c][:], in_=wkt[c])
                nc.sync.dma_start(out=wq_t[c][:], in_=wqt[c])
            bv_row = pp.tile([1, HD], BF16, tag="bvrow")
            bk_row = pp.tile([1, HD], BF16, tag="bkrow")
            nc.sync.dma_start(out=bv_row[:], in_=bvr[:])
            nc.sync.dma_start(out=bk_row[:], in_=bkr[:])
            bq_col = [pp.tile([128, 1], F32, tag=f"bqc{h}", name=f"bqc{h}")
                      for h in range(2)]
            for h in range(2):
                nc.sync.dma_start(out=bq_col[h][:], in_=bqc[h])
            i4 = [pp.tile([128, 4], BF16, tag=f"i4{h}", name=f"i4{h}")
                  for h in range(2)]
            nc.sync.dma_start(out=i4[0][:], in_=i4a_d[:])
            nc.sync.dma_start(out=i4[1][:], in_=i4b_d[:])
            ibcb_sb = [[pp.tile([32, 128], BF16, tag=f"ibcb{b}_{i}",
                                name=f"ibcb{b}_{i}") for i in range(2)]
                       for b in range(B)]
            for b in range(B):
                for i in range(2):
                    nc.sync.dma_start(out=ibcb_sb[b][i][:], in_=ibcb_d[b, i])
            ones_row = pp.tile([1, S], BF16, tag="ones_row")
            ones_col = pp.tile([128, 1], BF16, tag="ones_col")
            nc.sync.dma_start(out=ones_row[:], in_=ones_r_d[:])
            nc.sync.dma_start(out=ones_col[:], in_=ones_c_d[:])
            rrs_row = pp.tile([1, S], BF16, tag="rrs_row")
            nc.sync.dma_start(out=rrs_row[:], in_=rrs[:])

            # xs and xq resident for the whole kernel
            xs_sb = [[pp.tile([128, S], BF16, tag=f"xs{b}_{c}",
                              name=f"xs{b}_{c}") for c in range(2)]
                     for b in range(B)]
            xq_sb = [[pp.tile([128, S], BF16, tag=f"xq{b}_{c}",
                              name=f"xq{b}_{c}") for c in range(2)]
                     for b in range(B)]
            for b in range(B):
                for c in range(2):
                    nc.sync.dma_start(out=xs_sb[b][c][:], in_=xs[b, c])
            for b in range(B):
                for c in range(2):
                    nc.sync.dma_start(out=xq_sb[b][c][:], in_=xq[b, c])
            # adjT resident (fp8, 2.1 MB)
            adj_sb = [pp.tile([128, 2 * S], FP8, tag=f"adj{m}", name=f"adj{m}")
                      for m in range(16)]
            for m in range(16):
                nc.sync.dma_start(out=adj_sb[m][:], in_=adjt[m])

            # persistent SBUF tensors
            vt_sb = [[pp.tile([128, HD + 1], BF16, tag=f"vt{b}_{s}",
                              name=f"vt{b}_{s}") for s in range(4)]
                     for b in range(B)]
            q_sb = [[pp.tile([128, S], BF16, tag=f"q{b}_{h}", name=f"q{b}_{h}")
                     for h in range(2)] for b in range(B)]
            qsq_sb = [[pp.tile([128, S], BF16, tag=f"qq{b}_{h}",
                               name=f"qq{b}_{h}") for h in range(2)]
                      for b in range(B)]
            sq4N_all = pp.tile([32, S], F32, tag="sq4N", name="sq4N")
            t4_all = pp.tile([32, S], F32, tag="t4", name="t4")
            u4_all = pp.tile([32, S], F32, tag="u4", name="u4")
            w1_bf = pp.tile([32, S], BF16, tag="w1bf", name="w1bf")
            w2_bf = pp.tile([32, S], BF16, tag="w2bf", name="w2bf")
            vsp_all = pp.tile([4, B * D], BF16, tag="vspall", name="vspall")
            attn_sb = [pp.tile([128, S], F32, tag=f"at{p}", name=f"at{p}")
                       for p in range(4)]
            rrs_bc = pp.tile([128, S], F32, tag="rrs_bc")

            # ============ phase V: V projections + vmean (sb-major) ========
            with tc.tile_pool(name="psA", bufs=1, space="PSUM") as psA:
                prr = psA.tile([128, S], F32, tag="prr")
                nc.tensor.matmul(prr[:], lhsT=ones_row[:, 0:128],
                                 rhs=rrs_row[:], start=True, stop=True)
                nc.scalar.activation(rrs_bc[:], prr[:], ACTF.Copy)

                for sb in range(4):
                    sl = slice(sb * 128, (sb + 1) * 128)
                    for b in range(B):
                        pv = psA.tile([128, HD], F32, tag="pv", bufs=3)
                        nc.tensor.matmul(pv[:], lhsT=xs_sb[b][0][:, sl],
                                         rhs=wv_t[0][:], start=True,
                                         stop=False)
                        nc.tensor.matmul(pv[:], lhsT=xs_sb[b][1][:, sl],
                                         rhs=wv_t[1][:], start=False,
                                         stop=False)
                        nc.tensor.matmul(pv[:], lhsT=ones_row[:, 0:128],
                                         rhs=bv_row[:], start=False, stop=True)
                        vt = vt_sb[b][sb]
                        nc.scalar.activation(vt[:, 0:HD], pv[:], ACTF.Copy)
                        nc.vector.memset(vt[:, HD:HD + 1], 1.0)
                        vm_t = wk.tile([128, D], FP8, tag="vmt", bufs=2)
                        nc.vector.reduce_sum(
                            vm_t[:],
                            pv[:].rearrange("p (h d) -> p d h", h=H), axis=AX)
                        nc.sync.dma_start(
                            out=vm_loc[sb][:, b * D:(b + 1) * D], in_=vm_t[:])

            # AllGathers first (inputs ready earliest), AllReduce later
            for j in range(4):
                nc.gpsimd.collective_compute(
                    "AllGather", ALU.bypass, ins=[vm_loc[j].opt()],
                    outs=[vm_all[j].opt()], replica_groups=RG)

            # ============ phase K: K proj + norm + kvs (pipelined) =========
            with tc.tile_pool(name="psB", bufs=1, space="PSUM") as psB:
                kt_hist = {}

                def k_proj(b):
                    kt_t = []
                    for sb in range(4):
                        sl = slice(sb * 128, (sb + 1) * 128)
                        pk = psB.tile([128, HD], F32, tag="pk", bufs=3)
                        nc.tensor.matmul(pk[:], lhsT=xs_sb[b][0][:, sl],
                                         rhs=wk_t[0][:], start=True,
                                         stop=False)
                        nc.tensor.matmul(pk[:], lhsT=xs_sb[b][1][:, sl],
                                         rhs=wk_t[1][:], start=False,
                                         stop=False)
                        nc.tensor.matmul(pk[:], lhsT=ones_row[:, 0:128],
                                         rhs=bk_row[:], start=False, stop=True)
                        ksq = wk.tile([128, HD], F32, tag="ksq", bufs=2)
                        nc.scalar.activation(ksq[:], pk[:], ACTF.Square)
                        ssk = wk.tile([128, H], F32, tag="ssk", bufs=2)
                        nc.vector.reduce_sum(
                            ssk[:], ksq[:].rearrange("p (h d) -> p h d", h=H),
                            axis=AX)
                        snk = wk.tile([128, H], F32, tag="snk", bufs=2)
                        nc.scalar.activation(snk[:], ssk[:], ACTF.Sqrt)
                        rk = wk.tile([128, H], F32, tag="rk", bufs=2)
                        nc.vector.reciprocal(rk[:], snk[:])
                        kt = wk.tile([128, HD], BF16, tag=f"kt{sb}", bufs=2)
                        nc.vector.tensor_mul(
                            kt[:].rearrange("p (h d) -> p h d", h=H),
                            pk[:].rearrange("p (h d) -> p h d", h=H),
                            rk[:].broadcast_to((128, H, D)))
                        kt_t.append(kt)
                    kt_hist[b] = kt_t

                def kvs_phase(b):
                    kt_t = kt_hist.pop(b)
                    kvs0 = psB.tile([128, HD + 1], F32, tag="kvs0", bufs=1)
                    kvs1 = psB.tile([128, HD + 1], F32, tag="kvs1", bufs=1)
                    vs_ps = psB.tile([1, HD + 1], F32, tag="vsps", bufs=1)
                    for sb in range(4):
                        nc.tensor.matmul(kvs0[:], lhsT=kt_t[sb][:, 0:128],
                                         rhs=vt_sb[b][sb][:],
                                         start=(sb == 0), stop=(sb == 3))
                    for sb in range(4):
                        nc.tensor.matmul(kvs1[:], lhsT=kt_t[sb][:, 128:HD],
                                         rhs=vt_sb[b][sb][:],
                                         start=(sb == 0), stop=(sb == 3))
                    for sb in range(4):
                        nc.tensor.matmul(vs_ps[:], lhsT=ones_col[:],
                                         rhs=vt_sb[b][sb][:],
                                         start=(sb == 0), stop=(sb == 3))
                    for i, kvs in ((0, kvs0), (1, kvs1)):
                        pkx = wk.tile([128, 66], BF16, tag=f"pkx{i}", bufs=2)
                        nc.scalar.activation(pkx[0:64, 0:64],
                                             kvs[0:64, 128 * i:128 * i + 64],
                                             ACTF.Copy)
                        nc.scalar.activation(
                            pkx[64:128, 0:64],
                            kvs[64:128, 128 * i + 64:128 * i + 128], ACTF.Copy)
                        nc.vector.memset(pkx[:, 64:66], 0.0)
                        # ks columns scaled by 4 (folds the 4*t of u4)
                        nc.scalar.activation(pkx[0:64, 64:65],
                                             kvs[0:64, HD:HD + 1], ACTF.Copy,
                                             scale=4.0)
                        nc.scalar.activation(pkx[64:128, 65:66],
                                             kvs[64:128, HD:HD + 1], ACTF.Copy,
                                             scale=4.0)
                        nc.sync.dma_start(out=ar_in[b, i, 0:128, :],
                                          in_=pkx[:])
                    ksvs = wk.tile([1, 264], BF16, tag="ksvs", bufs=2)
                    nc.vector.memset(ksvs[:], 0.0)
                    for h in range(H):
                        nc.scalar.activation(
                            ksvs[0:1, 66 * h:66 * h + 64],
                            vs_ps[0:1, 64 * h:64 * h + 64], ACTF.Copy)
                    nc.sync.dma_start(out=ar_in[b, 0, 128:130, :],
                                      in_=ksvs[0:1, 0:132])
                    nc.sync.dma_start(out=ar_in[b, 1, 128:130, :],
                                      in_=ksvs[0:1, 132:264])

                for b in range(B):
                    k_proj(b)
                    if b > 0:
                        kvs_phase(b - 1)
                kvs_phase(B - 1)

            nc.gpsimd.collective_compute(
                "AllReduce", ALU.add, ins=[ar_in.opt()],
                outs=[ar_out.opt()], replica_groups=RG)

            # ============ phase Q (weight-stationary) ======================
            with tc.tile_pool(name="psC", bufs=1, space="PSUM") as psC:
                for half in range(2):
                    hsl = slice(half * 128, (half + 1) * 128)
                    for bb in (0, 4):
                        pqs = [psC.tile([128, S], F32, tag=f"pq{i}",
                                        name=f"pq{i}", bufs=1)
                               for i in range(4)]
                        for c in range(2):
                            for i in range(4):
                                nc.tensor.matmul(
                                    pqs[i][:], lhsT=wq_t[c][:, hsl],
                                    rhs=xq_sb[bb + i][c][:],
                                    start=(c == 0), stop=(c == 1))
                        for i in range(4):
                            b = bb + i
                            nc.scalar.activation(q_sb[b][half][:], pqs[i][:],
                                                 ACTF.Identity,
                                                 bias=bq_col[half][:])
                            nc.gpsimd.tensor_mul(qsq_sb[b][half][:],
                                                 q_sb[b][half][:],
                                                 q_sb[b][half][:])
                for b in range(B):
                    ssp = psC.tile([4, S], F32, tag="ss", bufs=2)
                    nc.tensor.matmul(ssp[:], lhsT=i4[0][:],
                                     rhs=qsq_sb[b][0][:], start=True,
                                     stop=False)
                    nc.tensor.matmul(ssp[:], lhsT=i4[1][:],
                                     rhs=qsq_sb[b][1][:], start=False,
                                     stop=True)
                    sqb = wk.tile([4, S], F32, tag="sqb", bufs=2)
                    nc.scalar.activation(sqb[:], ssp[:],
                                         ACTF.Sqrt, scale=float(16 * N * N))
                    nc.sync.dma_start(out=sq4N_all[4 * b:4 * b + 4, :],
                                      in_=sqb[:])

            # ============ GCN (DoubleRow fp8) + attention epilogue =========
            with tc.tile_pool(name="psD", bufs=1, space="PSUM") as psD:
                pg = [psD.tile([128, S], F32, tag=f"g{p}", name=f"g{p}")
                      for p in range(4)]
                if USE_DOUBLE_ROW:
                    for jp in range(2):
                        for peer in range(NCORES):
                            vmt = wk.tile([128, 2 * B * D], FP8, tag="vml",
                                          bufs=3)
                            nc.sync.dma_start(out=vmt[:, 0:512],
                                              in_=vm_all[2 * jp][peer])
                            nc.sync.dma_start(out=vmt[:, 512:1024],
                                              in_=vm_all[2 * jp + 1][peer])
                            vmv = vmt[:].rearrange("p (o f) -> p o f", o=2)
                            adv = adj_sb[peer * 2 + jp][:].rearrange(
                                "p (o f) -> p o f", o=2)
                            for p in range(4):
                                nc.tensor.matmul(
                                    pg[p][:],
                                    lhsT=vmv[:, :, 128 * p:128 * (p + 1)],
                                    rhs=adv[:],
                                    start=(jp == 0 and peer == 0),
                                    stop=(jp == 1 and peer == 7),
                                    perf_mode=PERF.DoubleRow)
                else:
                    for j in range(4):
                        for peer in range(NCORES):
                            vmt = wk.tile([128, B * D], FP8, tag="vml", bufs=3)
                            nc.sync.dma_start(out=vmt[:], in_=vm_all[j][peer])
                            adv = adj_sb[peer * 2 + j // 2][:].rearrange(
                                "p (o f) -> p o f", o=2)
                            for p in range(4):
                                nc.tensor.matmul(
                                    pg[p][:],
                                    lhsT=vmt[:, 128 * p:128 * (p + 1)],
                                    rhs=adv[:, j % 2, :],
                                    start=(j == 0 and peer == 0),
                                    stop=(j == 3 and peer == 7))

                # ---- attention epilogue (AllReduce landed during GCN) ----
                kpx_sb = [[None] * 2 for _ in range(B)]
                for b in range(B):
                    for i in range(2):
                        kpx = wk.tile([128, 66], BF16, tag=f"kpx{b}_{i}",
                                      bufs=1)
                        nc.sync.dma_start(out=kpx[:],
                                          in_=ar_out[b, i, 0:128, :])
                        kpx_sb[b][i] = kpx
                    for i in range(2):
                        pden = psD.tile([2, S], F32, tag="pden", bufs=2)
                        nc.tensor.matmul(pden[:],
                                         lhsT=kpx_sb[b][i][:, 64:66],
                                         rhs=q_sb[b][i][:], start=True,
                                         stop=True)
                        tt = wk.tile([2, S], F32, tag="tt", bufs=2)
                        nc.scalar.activation(tt[:], pden[:], ACTF.Copy)
                        nc.sync.dma_start(
                            out=t4_all[4 * b + 2 * i:4 * b + 2 * i + 2, :],
                            in_=tt[:])
                vspf = wk.tile([4, B * D], BF16, tag="vspf", bufs=1)
                for i in range(2):
                    nc.sync.dma_start(
                        out=vspf[2 * i:2 * i + 2, :],
                        in_=ar_out[:, i, 128:130, 0:64].rearrange(
                            "b r d -> r b d"))
                nc.scalar.activation(vsp_all[:], vspf[:], ACTF.Copy,
                                     scale=float(1.0 / (4 * N)))
                nc.vector.tensor_add(u4_all[:], t4_all[:], sq4N_all[:])
                w1f = wk.tile([32, S], F32, tag="w1f", bufs=1)
                nc.vector.reciprocal_approx_fast(w1f[:], u4_all[:])
                nc.scalar.activation(w1_bf[:], w1f[:], ACTF.Copy)
                nc.vector.tensor_mul(w2_bf[:], w1f[:], sq4N_all[:])

                for b in range(B):
                    w2b = wk.tile([4, S], BF16, tag="w2b", bufs=2)
                    nc.sync.dma_start(out=w2b[:],
                                      in_=w2_bf[4 * b:4 * b + 4, :])
                    qs_t = []
                    for i in range(2):
                        pbc = psD.tile([128, S], F32, tag="pbc", bufs=1)
                        nc.tensor.matmul(pbc[:], lhsT=ibcb_sb[b][i][:],
                                         rhs=w1_bf[:], start=True, stop=True)
                        qs = wk.tile([128, S], BF16, tag=f"qs{i}", bufs=2)
                        nc.vector.tensor_mul(qs[:], q_sb[b][i][:], pbc[:])
                        qs_t.append(qs)
                    pat = psD.tile([D, S], F32, tag="pat", bufs=1)
                    nc.tensor.matmul(pat[:], lhsT=kpx_sb[b][0][:, 0:64],
                                     rhs=qs_t[0][:], start=True, stop=False)
                    nc.tensor.matmul(pat[:], lhsT=kpx_sb[b][1][:, 0:64],
                                     rhs=qs_t[1][:], start=False, stop=False)
                    nc.tensor.matmul(pat[:],
                                     lhsT=vsp_all[:, b * D:(b + 1) * D],
                                     rhs=w2b[:], start=False, stop=True)
                    nc.scalar.activation(
                        attn_sb[b // 2][(b % 2) * D:(b % 2 + 1) * D, :],
                        pat[:], ACTF.Copy)

                for p in range(4):
                    gt = wk.tile([128, S], F32, tag="gt", bufs=2)
                    nc.vector.tensor_mul(gt[:], pg[p][:], rrs_bc[:])
                    ot = wk.tile([128, S], F32, tag="ot", bufs=2)
                    nc.vector.tensor_add(ot[:], gt[:], attn_sb[p][:])
                    nc.sync.dma_start(out=out[2 * p], in_=ot[0:D, :])
                    nc.sync.dma_start(out=out[2 * p + 1], in_=ot[D:128, :])
    nc.compile()
    return nc


def _prep_inputs(query_input, source_input, adj, Wq_w, Wq_b, Wk_w, Wk_b,
                 Wv_w, Wv_b):
    bf16 = ml_dtypes.bfloat16
    fp8 = ml_dtypes.float8_e4m3fn
    xq_np = np.asarray(query_input, dtype=np.float32)
    xs_np = np.asarray(source_input, dtype=np.float32)
    adj_np = np.asarray(adj, dtype=np.float32)

    adjT = np.ascontiguousarray(adj_np.T)
    np.fill_diagonal(adjT, adjT.diagonal() + 1.0)
    adjT_f8 = adjT.astype(fp8)
    rrs_full = (0.25 / (adj_np.sum(axis=1) + 1.0)).astype(bf16)

    wvt = np.ascontiguousarray(np.asarray(Wv_w, np.float32).T)
    wvt = wvt.astype(bf16).reshape(2, 128, HD)
    wkt = np.ascontiguousarray(np.asarray(Wk_w, np.float32).T)
    wkt = wkt.astype(bf16).reshape(2, 128, HD)
    wqt = np.ascontiguousarray(np.asarray(Wq_w, np.float32).T)
    wqt = wqt.astype(bf16).reshape(2, 128, HD)
    bvr = np.asarray(Wv_b, np.float32).astype(bf16).reshape(1, HD)
    bkr = np.asarray(Wk_b, np.float32).astype(bf16).reshape(1, HD)
    bqc = np.asarray(Wq_b, np.float32).reshape(2, 128, 1)

    i4a, i4b, ibcb = _indicators()
    in_maps = []
    for i in range(NCORES):
        sl = slice(i * S, (i + 1) * S)
        # [4096, S] -> [peer, jp, o, ki, n] -> [peer, jp, ki, o, n]
        a = adjT_f8[:, sl].reshape(8, 2, 2, 128, S)
        a = np.ascontiguousarray(a.transpose(0, 1, 3, 2, 4))
        in_maps.append({
            "xq": np.ascontiguousarray(xq_np[:, :, sl]).astype(bf16)
                  .reshape(B, 2, 128, S),
            "xs": np.ascontiguousarray(xs_np[:, :, sl]).astype(bf16)
                  .reshape(B, 2, 128, S),
            "adjt": a.reshape(16, 128, 2 * S),
            "rrs": np.ascontiguousarray(rrs_full[sl]).reshape(1, S),
            "wvt": wvt, "wkt": wkt, "wqt": wqt,
            "bvr": bvr, "bkr": bkr, "bqc": bqc,
            "i4a_in": i4a.astype(bf16), "i4b_in": i4b.astype(bf16),
            "ibcb_in": ibcb.astype(bf16),
            "ones_r": np.ones((1, S), bf16),
            "ones_c": np.ones((128, 1), bf16),
        })
    return in_maps


def kernel(**inputs):
    if "nc" not in _CACHE:
        _CACHE["nc"] = _build()
    nc = _CACHE["nc"]
    in_maps = _prep_inputs(**inputs)
    res = run_bass_kernel_spmd(nc, in_maps, list(range(NCORES)))
    full = np.empty((B, D, N), np.float32)
    for i in range(NCORES):
        full[:, :, i * S:(i + 1) * S] = res.results[i]["out"]
    return full


# revision 29
# speedup vs baseline: 2.2368x; 1.2246x over previous
"""DIFFormerConv (simple linear attention + dense GCN) on 8 trn2 NeuronCores.

Sharding: nodes N=4096 split 8 ways (S=512 per core). Phase order is chosen
so both collectives hide under compute:
  V-pass (vmean -> fp8)  -> 4 chunked AllGathers start ~15% in
  K-pass + kvs partials  -> bf16 AllReduce (kvs diag blocks | ks cols | vs)
  Q-pass (weight-stationary, bf16)
  GCN (DoubleRow fp8: adjT resident in SBUF, vmean pairs as lhsT)
  attention epilogue (needs AllReduce, which landed during GCN)
  combine + store

Layouts (no PE transposes anywhere):
  q:   [hd, s]   (heads*dim on partitions)  -- lhsT = Wq^T chunks (stationary)
  k,v: [s, hd]   (transposed projection)    -- lhsT = xs chunks (stationary)
  kvs: lhsT = kt chunk, rhs = [vt | ones]   -- ks falls out as PSUM column 256
  gcn: [(b,d), n] -- lhsT = vm pairs [128,2,128] fp8, rhs = adjT [128,2,512]
Denominator algebra (one stacked [32,S] approx reciprocal):
  w1 = 1/(4*t + 4*N*sqrt(ss)),  w2 = 4*N*sqrt(ss)*w1,  t = q . ks (raw q)
  attn = sum_h kvs_h^T @ (q_h * w1_h) + (vs/(4N))^T @ w2
The PE on this part runs at 1.2 GHz regardless of HAM, so MM cycles are
minimized (DoubleRow fp8 GCN, N>=256 everywhere, dense issue order).
"""

import sys

sys.path.insert(0, "/opt/trn_rl_repo")

import numpy as np
import ml_dtypes

from concourse import bass, bacc, tile, mybir
from concourse.bass_utils import run_bass_kernel_spmd

B, C, N, H, D = 8, 256, 4096, 4, 64
NCORES = 8
S = N // NCORES          # 512 nodes per core
HD = H * D               # 256
F32 = mybir.dt.float32
BF16 = mybir.dt.bfloat16
FP8 = mybir.dt.float8e4
AX = mybir.AxisListType.X
ALU = mybir.AluOpType
ACTF = mybir.ActivationFunctionType
PERF = mybir.MatmulPerfMode
RG = [list(range(NCORES))]

USE_DOUBLE_ROW = True

_CACHE = {}


def _indicators():
    i4a = np.zeros((128, 4), np.float32)
    i4b = np.zeros((128, 4), np.float32)
    for p in range(128):
        i4a[p, p // 64] = 1.0
        i4b[p, 2 + p // 64] = 1.0
    ibcb = np.zeros((B, 2, 32, 128), np.float32)
    for b in range(B):
        for i in range(2):
            for p in range(128):
                ibcb[b, i, 4 * b + 2 * i + p // 64, p] = 1.0
    return i4a, i4b, ibcb


def _build():
    nc = bacc.Bacc("TRN2", target_bir_lowering=False, debug=False,
                   num_devices=NCORES)

    xq = nc.dram_tensor("xq", [B, 2, 128, S], BF16, kind="ExternalInput")
    xs = nc.dram_tensor("xs", [B, 2, 128, S], BF16, kind="ExternalInput")
    # DoubleRow layout: [peer*2+jp, ki, o, n] = adjT[peer*512+jp*256+o*128+ki, n]
    adjt = nc.dram_tensor("adjt", [16, 128, 2 * S], FP8, kind="ExternalInput")
    rrs = nc.dram_tensor("rrs", [1, S], BF16, kind="ExternalInput")
    wkv = nc.dram_tensor("wkv", [2, 128, 2 * HD], BF16, kind="ExternalInput")
    bkv = nc.dram_tensor("bkv", [1, 2 * HD], BF16, kind="ExternalInput")
    wqt = nc.dram_tensor("wqt", [2, 128, HD], BF16, kind="ExternalInput")
    bqc = nc.dram_tensor("bqc", [2, 128, 1], F32, kind="ExternalInput")
    i4a_d = nc.dram_tensor("i4a_in", [128, 4], BF16, kind="ExternalInput")
    i4b_d = nc.dram_tensor("i4b_in", [128, 4], BF16, kind="ExternalInput")
    ibcb_d = nc.dram_tensor("ibcb_in", [B, 2, 32, 128], BF16,
                            kind="ExternalInput")
    ones_r_d = nc.dram_tensor("ones_r", [1, S], BF16, kind="ExternalInput")
    ones_c_d = nc.dram_tensor("ones_c", [128, 1], BF16, kind="ExternalInput")
    out = nc.dram_tensor("out", [B, D, S], F32, kind="ExternalOutput")

    with nc.allow_low_precision(reason="bf16/fp8 rounding intentional"), \
            tile.TileContext(nc) as tc:
        with (
            tc.tile_pool(name="pers", bufs=1) as pp,
            tc.tile_pool(name="work", bufs=3) as wk,
            tc.tile_pool(name="dram", bufs=1, space="DRAM") as dp,
        ):
            vm_loc = [dp.tile([128, B * D], FP8, tag=f"vml{j}",
                              name=f"vml{j}") for j in range(4)]
            vm_all = [dp.tile([NCORES, 128, B * D], FP8, tag=f"vma{j}",
                              name=f"vma{j}", addr_space="Shared")
                      for j in range(4)]
            ar_in = dp.tile([B, 2, 130, 66], BF16, tag="ar_in", name="ar_in")
            ar_out = dp.tile([B, 2, 130, 66], BF16, tag="ar_out",
                             name="ar_out", addr_space="Shared")

            # ---- constants ----
            wkv_t = [pp.tile([128, 2 * HD], BF16, tag=f"wkv{c}",
                             name=f"wkv{c}") for c in range(2)]
            wq_t = [pp.tile([128, HD], BF16, tag=f"wq{c}", name=f"wq{c}")
                    for c in range(2)]
            for c in range(2):
                nc.sync.dma_start(out=wkv_t[c][:], in_=wkv[c])
                nc.sync.dma_start(out=wq_t[c][:], in_=wqt[c])
            bkv_row = pp.tile([1, 2 * HD], BF16, tag="bkvrow")
            nc.sync.dma_start(out=bkv_row[:], in_=bkv[:])
            bq_col = [pp.tile([128, 1], F32, tag=f"bqc{h}", name=f"bqc{h}")
                      for h in range(2)]
            for h in range(2):
                nc.sync.dma_start(out=bq_col[h][:], in_=bqc[h])
            i4 = [pp.tile([128, 4], BF16, tag=f"i4{h}", name=f"i4{h}")
                  for h in range(2)]
            nc.sync.dma_start(out=i4[0][:], in_=i4a_d[:])
            nc.sync.dma_start(out=i4[1][:], in_=i4b_d[:])
            ibcb_sb = [[pp.tile([32, 128], BF16, tag=f"ibcb{b}_{i}",
                                name=f"ibcb{b}_{i}") for i in range(2)]
                       for b in range(B)]
            for b in range(B):
                for i in range(2):
                    nc.sync.dma_start(out=ibcb_sb[b][i][:], in_=ibcb_d[b, i])
            ones_row = pp.tile([1, S], BF16, tag="ones_row")
            ones_col = pp.tile([128, 1], BF16, tag="ones_col")
            nc.sync.dma_start(out=ones_row[:], in_=ones_r_d[:])
            nc.sync.dma_start(out=ones_col[:], in_=ones_c_d[:])
            rrs_row = pp.tile([1, S], BF16, tag="rrs_row")
            nc.sync.dma_start(out=rrs_row[:], in_=rrs[:])

            # xs and xq resident for the whole kernel
            xs_sb = [[pp.tile([128, S], BF16, tag=f"xs{b}_{c}",
                              name=f"xs{b}_{c}") for c in range(2)]
                     for b in range(B)]
            xq_sb = [[pp.tile([128, S], BF16, tag=f"xq{b}_{c}",
                              name=f"xq{b}_{c}") for c in range(2)]
                     for b in range(B)]
            for b in range(B):
                for c in range(2):
                    nc.sync.dma_start(out=xs_sb[b][c][:], in_=xs[b, c])
            # xq/adjT preloads are issued AFTER the collectives (below) so
            # they don't delay the xs tiles feeding the first projections
            adj_sb = [pp.tile([128, 2 * S], FP8, tag=f"adj{m}", name=f"adj{m}")
                      for m in range(16)]

            # persistent SBUF tensors
            vt_sb = [[pp.tile([128, HD + 1], BF16, tag=f"vt{b}_{s}",
                              name=f"vt{b}_{s}") for s in range(4)]
                     for b in range(B)]
            q_sb = [[pp.tile([128, S], BF16, tag=f"q{b}_{h}", name=f"q{b}_{h}")
                     for h in range(2)] for b in range(B)]
            qsq_sb = [[pp.tile([128, S], BF16, tag=f"qq{b}_{h}",
                               name=f"qq{b}_{h}") for h in range(2)]
                      for b in range(B)]
            sq4N_all = pp.tile([32, S], F32, tag="sq4N", name="sq4N")
            t4_all = pp.tile([32, S], F32, tag="t4", name="t4")
            u4_all = pp.tile([32, S], F32, tag="u4", name="u4")
            w1_bf = pp.tile([32, S], BF16, tag="w1bf", name="w1bf")
            w2_bf = pp.tile([32, S], BF16, tag="w2bf", name="w2bf")
            vsp_all = pp.tile([4, B * D], BF16, tag="vspall", name="vspall")
            attn_sb = [pp.tile([128, S], F32, tag=f"at{p}", name=f"at{p}")
                       for p in range(4)]
            rrs_bc = pp.tile([128, S], F32, tag="rrs_bc")

            # ===== phase VK: concat K|V projection (sb-major) + kvs =====
            with tc.tile_pool(name="psA", bufs=1, space="PSUM") as psA:
                prr = psA.tile([128, S], F32, tag="prr")
                nc.tensor.matmul(prr[:], lhsT=ones_row[:, 0:128],
                                 rhs=rrs_row[:], start=True, stop=True)
                nc.scalar.activation(rrs_bc[:], prr[:], ACTF.Copy)

                kt_hist = {}

                def kv_proj(b, sb):
                    sl = slice(sb * 128, (sb + 1) * 128)
                    pkv = psA.tile([128, 2 * HD], F32, tag="pkv", bufs=3)
                    nc.tensor.matmul(pkv[:], lhsT=xs_sb[b][0][:, sl],
                                     rhs=wkv_t[0][:], start=True, stop=False)
                    nc.tensor.matmul(pkv[:], lhsT=xs_sb[b][1][:, sl],
                                     rhs=wkv_t[1][:], start=False, stop=False)
                    nc.tensor.matmul(pkv[:], lhsT=ones_row[:, 0:128],
                                     rhs=bkv_row[:], start=False, stop=True)
                    vt = vt_sb[b][sb]
                    nc.scalar.activation(vt[:, 0:HD], pkv[:, HD:2 * HD],
                                         ACTF.Copy)
                    nc.vector.memset(vt[:, HD:HD + 1], 1.0)
                    vm_t = wk.tile([128, D], FP8, tag="vmt", bufs=2)
                    nc.vector.reduce_sum(
                        vm_t[:],
                        pkv[:, HD:2 * HD].rearrange("p (h d) -> p d h", h=H),
                        axis=AX)
                    nc.sync.dma_start(
                        out=vm_loc[sb][:, b * D:(b + 1) * D], in_=vm_t[:])
                    ksq = wk.tile([128, HD], F32, tag="ksq", bufs=2)
                    nc.scalar.activation(ksq[:], pkv[:, 0:HD], ACTF.Square)
                    ssk = wk.tile([128, H], F32, tag="ssk", bufs=2)
                    nc.vector.reduce_sum(
                        ssk[:], ksq[:].rearrange("p (h d) -> p h d", h=H),
                        axis=AX)
                    snk = wk.tile([128, H], F32, tag="snk", bufs=2)
                    nc.scalar.activation(snk[:], ssk[:], ACTF.Sqrt)
                    rk = wk.tile([128, H], F32, tag="rk", bufs=2)
                    nc.vector.reciprocal(rk[:], snk[:])
                    kt = wk.tile([128, HD], BF16, tag=f"kt{b}_{sb}",
                                 name=f"kt{b}_{sb}", bufs=1)
                    nc.vector.tensor_mul(
                        kt[:].rearrange("p (h d) -> p h d", h=H),
                        pkv[:, 0:HD].rearrange("p (h d) -> p h d", h=H),
                        rk[:].broadcast_to((128, H, D)))
                    kt_hist.setdefault(b, []).append(kt)

                def kvs_phase(b):
                    kt_t = kt_hist.pop(b)
                    kvs0 = psA.tile([128, HD + 1], F32, tag="kvs0", bufs=1)
                    kvs1 = psA.tile([128, HD + 1], F32, tag="kvs1", bufs=1)
                    vs_ps = psA.tile([1, HD + 1], F32, tag="vsps", bufs=1)
                    for sb in range(4):
                        nc.tensor.matmul(kvs0[:], lhsT=kt_t[sb][:, 0:128],
                                         rhs=vt_sb[b][sb][:],
                                         start=(sb == 0), stop=(sb == 3))
                    for sb in range(4):
                        nc.tensor.matmul(kvs1[:], lhsT=kt_t[sb][:, 128:HD],
                                         rhs=vt_sb[b][sb][:],
                                         start=(sb == 0), stop=(sb == 3))
                    for sb in range(4):
                        nc.tensor.matmul(vs_ps[:], lhsT=ones_col[:],
                                         rhs=vt_sb[b][sb][:],
                                         start=(sb == 0), stop=(sb == 3))
                    for i, kvs in ((0, kvs0), (1, kvs1)):
                        pkx = wk.tile([128, 66], BF16, tag=f"pkx{i}", bufs=2)
                        nc.scalar.activation(pkx[0:64, 0:64],
                                             kvs[0:64, 128 * i:128 * i + 64],
                                             ACTF.Copy)
                        nc.scalar.activation(
                            pkx[64:128, 0:64],
                            kvs[64:128, 128 * i + 64:128 * i + 128], ACTF.Copy)
                        nc.vector.memset(pkx[:, 64:66], 0.0)
                        # ks columns scaled by 4 (folds the 4*t of u4)
                        nc.scalar.activation(pkx[0:64, 64:65],
                                             kvs[0:64, HD:HD + 1], ACTF.Copy,
                                             scale=4.0)
                        nc.scalar.activation(pkx[64:128, 65:66],
                                             kvs[64:128, HD:HD + 1], ACTF.Copy,
                                             scale=4.0)
                        nc.sync.dma_start(out=ar_in[b, i, 0:128, :],
                                          in_=pkx[:])
                    ksvs = wk.tile([1, 264], BF16, tag="ksvs", bufs=2)
                    nc.vector.memset(ksvs[:], 0.0)
                    for h in range(H):
                        nc.scalar.activation(
                            ksvs[0:1, 66 * h:66 * h + 64],
                            vs_ps[0:1, 64 * h:64 * h + 64], ACTF.Copy)
                    nc.sync.dma_start(out=ar_in[b, 0, 128:130, :],
                                      in_=ksvs[0:1, 0:132])
                    nc.sync.dma_start(out=ar_in[b, 1, 128:130, :],
                                      in_=ksvs[0:1, 132:264])

                ag_launched = 0
                for sb in range(4):
                    for b in range(B):
                        kv_proj(b, sb)
                        if sb == 3 and b > 0:
                            kvs_phase(b - 1)
                    # chunk sb complete: launch its AllGather immediately
                    nc.gpsimd.collective_compute(
                        "AllGather", ALU.bypass, ins=[vm_loc[sb].opt()],
                        outs=[vm_all[sb].opt()], replica_groups=RG)
                kvs_phase(B - 1)

            nc.gpsimd.collective_compute(
                "AllReduce", ALU.add, ins=[ar_in.opt()],
                outs=[ar_out.opt()], replica_groups=RG)

            # bulk preloads for the later phases (issued late so they don't
            # delay the xs tiles feeding the first projections)
            for b in range(B):
                for c in range(2):
                    nc.sync.dma_start(out=xq_sb[b][c][:], in_=xq[b, c])
            for m in range(16):
                nc.sync.dma_start(out=adj_sb[m][:], in_=adjt[m])

            # ============ phase Q (weight-stationary) ======================
            with tc.tile_pool(name="psC", bufs=1, space="PSUM") as psC:
                for half in range(2):
                    hsl = slice(half * 128, (half + 1) * 128)
                    for bb in (0, 4):
                        pqs = [psC.tile([128, S], F32, tag=f"pq{i}",
                                        name=f"pq{i}", bufs=1)
                               for i in range(4)]
                        for c in range(2):
                            for i in range(4):
                                nc.tensor.matmul(
                                    pqs[i][:], lhsT=wq_t[c][:, hsl],
                                    rhs=xq_sb[bb + i][c][:],
                                    start=(c == 0), stop=(c == 1))
                        for i in range(4):
                            b = bb + i
                            nc.scalar.activation(q_sb[b][half][:], pqs[i][:],
                                                 ACTF.Identity,
                                                 bias=bq_col[half][:])
                            nc.vector.tensor_mul(qsq_sb[b][half][:],
                                                 q_sb[b][half][:],
                                                 q_sb[b][half][:])
                for b in range(B):
                    ssp = psC.tile([4, S], F32, tag="ss", bufs=2)
                    nc.tensor.matmul(ssp[:], lhsT=i4[0][:],
                                     rhs=qsq_sb[b][0][:], start=True,
                                     stop=False)
                    nc.tensor.matmul(ssp[:], lhsT=i4[1][:],
                                     rhs=qsq_sb[b][1][:], start=False,
                                     stop=True)
                    sqb = wk.tile([4, S], F32, tag="sqb", bufs=2)
                    nc.scalar.activation(sqb[:], ssp[:],
                                         ACTF.Sqrt, scale=float(16 * N * N))
                    nc.sync.dma_start(out=sq4N_all[4 * b:4 * b + 4, :],
                                      in_=sqb[:])

            # ============ GCN (DoubleRow fp8) + attention epilogue =========
            with tc.tile_pool(name="psD", bufs=1, space="PSUM") as psD:
                pg = [psD.tile([128, S], F32, tag=f"g{p}", name=f"g{p}")
                      for p in range(4)]
                if USE_DOUBLE_ROW:
                    for jp in range(2):
                        for peer in range(NCORES):
                            vmt = wk.tile([128, 2 * B * D], FP8, tag="vml",
                                          bufs=3)
                            nc.sync.dma_start(out=vmt[:, 0:512],
                                              in_=vm_all[2 * jp][peer])
                            nc.sync.dma_start(out=vmt[:, 512:1024],
                                              in_=vm_all[2 * jp + 1][peer])
                            vmv = vmt[:].rearrange("p (o f) -> p o f", o=2)
                            adv = adj_sb[peer * 2 + jp][:].rearrange(
                                "p (o f) -> p o f", o=2)
                            for p in range(4):
                                nc.tensor.matmul(
                                    pg[p][:],
                                    lhsT=vmv[:, :, 128 * p:128 * (p + 1)],
                                    rhs=adv[:],
                                    start=(jp == 0 and peer == 0),
                                    stop=(jp == 1 and peer == 7),
                                    perf_mode=PERF.DoubleRow)
                else:
                    for j in range(4):
                        for peer in range(NCORES):
                            vmt = wk.tile([128, B * D], FP8, tag="vml", bufs=3)
                            nc.sync.dma_start(out=vmt[:], in_=vm_all[j][peer])
                            adv = adj_sb[peer * 2 + j // 2][:].rearrange(
                                "p (o f) -> p o f", o=2)
                            for p in range(4):
                                nc.tensor.matmul(
                                    pg[p][:],
                                    lhsT=vmt[:, 128 * p:128 * (p + 1)],
                                    rhs=adv[:, j % 2, :],
                                    start=(j == 0 and peer == 0),
                                    stop=(j == 3 and peer == 7))

                # ---- attention epilogue (AllReduce landed during GCN) ----
                kpx_sb = [[None] * 2 for _ in range(B)]
                for b in range(B):
                    for i in range(2):
                        kpx = wk.tile([128, 66], BF16, tag=f"kpx{b}_{i}",
                                      bufs=1)
                        nc.sync.dma_start(out=kpx[:],
                                          in_=ar_out[b, i, 0:128, :])
                        kpx_sb[b][i] = kpx
                    for i in range(2):
                        pden = psD.tile([2, S], F32, tag="pden", bufs=2)
                        nc.tensor.matmul(pden[:],
                                         lhsT=kpx_sb[b][i][:, 64:66],
                                         rhs=q_sb[b][i][:], start=True,
                                         stop=True)
                        tt = wk.tile([2, S], F32, tag="tt", bufs=2)
                        nc.scalar.activation(tt[:], pden[:], ACTF.Copy)
                        nc.sync.dma_start(
                            out=t4_all[4 * b + 2 * i:4 * b + 2 * i + 2, :],
                            in_=tt[:])
                vspf = wk.tile([4, B * D], BF16, tag="vspf", bufs=1)
                for i in range(2):
                    nc.sync.dma_start(
                        out=vspf[2 * i:2 * i + 2, :],
                        in_=ar_out[:, i, 128:130, 0:64].rearrange(
                            "b r d -> r b d"))
                nc.scalar.activation(vsp_all[:], vspf[:], ACTF.Copy,
                                     scale=float(1.0 / (4 * N)))
                nc.vector.tensor_add(u4_all[:], t4_all[:], sq4N_all[:])
                w1f = wk.tile([32, S], F32, tag="w1f", bufs=1)
                nc.vector.reciprocal_approx_fast(w1f[:], u4_all[:])
                nc.scalar.activation(w1_bf[:], w1f[:], ACTF.Copy)
                nc.vector.tensor_mul(w2_bf[:], w1f[:], sq4N_all[:])

                for b in range(B):
                    w2b = wk.tile([4, S], BF16, tag="w2b", bufs=2)
                    nc.sync.dma_start(out=w2b[:],
                                      in_=w2_bf[4 * b:4 * b + 4, :])
                    qs_t = []
                    for i in range(2):
                        pbc = psD.tile([128, S], F32, tag="pbc", bufs=1)
                        nc.tensor.matmul(pbc[:], lhsT=ibcb_sb[b][i][:],
                                         rhs=w1_bf[:], start=True, stop=True)
                        qs = wk.tile([128, S], BF16, tag=f"qs{i}", bufs=2)
                        nc.vector.tensor_mul(qs[:], q_sb[b][i][:], pbc[:])
                        qs_t.append(qs)
                    pat = psD.tile([D, S], F32, tag="pat", bufs=1)
                    nc.tensor.matmul(pat[:], lhsT=kpx_sb[b][0][:, 0:64],
                                     rhs=qs_t[0][:], start=True, stop=False)
                    nc.tensor.matmul(pat[:], lhsT=kpx_sb[b][1][:, 0:64],
                                     rhs=qs_t[1][:], start=False, stop=False)
                    nc.tensor.matmul(pat[:],
                                     lhsT=vsp_all[:, b * D:(b + 1) * D],
                                     rhs=w2b[:], start=False, stop=True)
                    nc.scalar.activation(
                        attn_sb[b // 2][(b % 2) * D:(b % 2 + 1) * D, :],
                        pat[:], ACTF.Copy)

                for p in range(4):
                    gt = wk.tile([128, S], F32, tag="gt", bufs=2)
                    nc.vector.tensor_mul(gt[:], pg[p][:], rrs_bc[:])
                    ot = wk.tile([128, S], F32, tag="ot", bufs=2)
                    nc.vector.tensor_add(ot[:], gt[:], attn_sb[p][:])
                    nc.sync.dma_start(out=out[2 * p], in_=ot[0:D, :])
                    nc.sync.dma_start(out=out[2 * p + 1], in_=ot[D:128, :])
    nc.compile()
    return nc


def _prep_inputs(query_input, source_input, adj, Wq_w, Wq_b, Wk_w, Wk_b,
                 Wv_w, Wv_b):
    bf16 = ml_dtypes.bfloat16
    fp8 = ml_dtypes.float8_e4m3fn
    xq_np = np.asarray(query_input, dtype=np.float32)
    xs_np = np.asarray(source_input, dtype=np.float32)
    adj_np = np.asarray(adj, dtype=np.float32)

    adjT = np.ascontiguousarray(adj_np.T)
    np.fill_diagonal(adjT, adjT.diagonal() + 1.0)
    adjT_f8 = adjT.astype(fp8)
    rrs_full = (0.25 / (adj_np.sum(axis=1) + 1.0)).astype(bf16)

    wkv_np = np.concatenate([np.asarray(Wk_w, np.float32).T,
                             np.asarray(Wv_w, np.float32).T], axis=1)
    wkv_np = np.ascontiguousarray(wkv_np).astype(bf16).reshape(2, 128, 2 * HD)
    bkv_np = np.concatenate([np.asarray(Wk_b, np.float32),
                             np.asarray(Wv_b, np.float32)])
    bkv_np = bkv_np.astype(bf16).reshape(1, 2 * HD)
    wqt = np.ascontiguousarray(np.asarray(Wq_w, np.float32).T)
    wqt = wqt.astype(bf16).reshape(2, 128, HD)
    bqc = np.asarray(Wq_b, np.float32).reshape(2, 128, 1)

    i4a, i4b, ibcb = _indicators()
    in_maps = []
    for i in range(NCORES):
        sl = slice(i * S, (i + 1) * S)
        # [4096, S] -> [peer, jp, o, ki, n] -> [peer, jp, ki, o, n]
        a = adjT_f8[:, sl].reshape(8, 2, 2, 128, S)
        a = np.ascontiguousarray(a.transpose(0, 1, 3, 2, 4))
        in_maps.append({
            "xq": np.ascontiguousarray(xq_np[:, :, sl]).astype(bf16)
                  .reshape(B, 2, 128, S),
            "xs": np.ascontiguousarray(xs_np[:, :, sl]).astype(bf16)
                  .reshape(B, 2, 128, S),
            "adjt": a.reshape(16, 128, 2 * S),
            "rrs": np.ascontiguousarray(rrs_full[sl]).reshape(1, S),
            "wkv": wkv_np, "bkv": bkv_np, "wqt": wqt, "bqc": bqc,
            "i4a_in": i4a.astype(bf16), "i4b_in": i4b.astype(bf16),
            "ibcb_in": ibcb.astype(bf16),
            "ones_r": np.ones((1, S), bf16),
            "ones_c": np.ones((128, 1), bf16),
        })
    return in_maps


def kernel(**inputs):
    if "nc" not in _CACHE:
        _CACHE["nc"] = _build()
    nc = _CACHE["nc"]
    in_maps = _prep_inputs(**inputs)
    res = run_bass_kernel_spmd(nc, in_maps, list(range(NCORES)))
    full = np.empty((B, D, N), np.float32)
    for i in range(NCORES):
        full[:, :, i * S:(i + 1) * S] = res.results[i]["out"]
    return full


# revision 30
# speedup vs baseline: 2.3546x; 1.0527x over previous
"""DIFFormerConv (simple linear attention + dense GCN) on 8 trn2 NeuronCores.

Sharding: nodes N=4096 split 8 ways (S=512 per core). Phase order is chosen
so both collectives hide under compute:
  V-pass (vmean -> fp8)  -> 4 chunked AllGathers start ~15% in
  K-pass + kvs partials  -> bf16 AllReduce (kvs diag blocks | ks cols | vs)
  Q-pass (weight-stationary, bf16)
  GCN (DoubleRow fp8: adjT resident in SBUF, vmean pairs as lhsT)
  attention epilogue (needs AllReduce, which landed during GCN)
  combine + store

Layouts (no PE transposes anywhere):
  q:   [hd, s]   (heads*dim on partitions)  -- lhsT = Wq^T chunks (stationary)
  k,v: [s, hd]   (transposed projection)    -- lhsT = xs chunks (stationary)
  kvs: lhsT = kt chunk, rhs = [vt | ones]   -- ks falls out as PSUM column 256
  gcn: [(b,d), n] -- lhsT = vm pairs [128,2,128] fp8, rhs = adjT [128,2,512]
Denominator algebra (one stacked [32,S] approx reciprocal):
  w1 = 1/(4*t + 4*N*sqrt(ss)),  w2 = 4*N*sqrt(ss)*w1,  t = q . ks (raw q)
  attn = sum_h kvs_h^T @ (q_h * w1_h) + (vs/(4N))^T @ w2
The PE on this part runs at 1.2 GHz regardless of HAM, so MM cycles are
minimized (DoubleRow fp8 GCN, N>=256 everywhere, dense issue order).
"""

import sys

sys.path.insert(0, "/opt/trn_rl_repo")

import numpy as np
import ml_dtypes

from concourse import bass, bacc, tile, mybir
from concourse.bass_utils import run_bass_kernel_spmd

B, C, N, H, D = 8, 256, 4096, 4, 64
NCORES = 8
S = N // NCORES          # 512 nodes per core
HD = H * D               # 256
F32 = mybir.dt.float32
BF16 = mybir.dt.bfloat16
FP8 = mybir.dt.float8e4
AX = mybir.AxisListType.X
ALU = mybir.AluOpType
ACTF = mybir.ActivationFunctionType
PERF = mybir.MatmulPerfMode
RG = [list(range(NCORES))]

USE_DOUBLE_ROW = True

_CACHE = {}


def _indicators():
    i4a = np.zeros((128, 4), np.float32)
    i4b = np.zeros((128, 4), np.float32)
    for p in range(128):
        i4a[p, p // 64] = 1.0
        i4b[p, 2 + p // 64] = 1.0
    ibcb = np.zeros((B, 2, 32, 128), np.float32)
    for b in range(B):
        for i in range(2):
            for p in range(128):
                ibcb[b, i, 4 * b + 2 * i + p // 64, p] = 1.0
    return i4a, i4b, ibcb


def _build():
    nc = bacc.Bacc("TRN2", target_bir_lowering=False, debug=False,
                   num_devices=NCORES)

    xq = nc.dram_tensor("xq", [B, 2, 128, S], BF16, kind="ExternalInput")
    xs = nc.dram_tensor("xs", [B, 2, 128, S], BF16, kind="ExternalInput")
    # DoubleRow layout: [peer*2+jp, ki, o, n] = adjT[peer*512+jp*256+o*128+ki, n]
    adjt = nc.dram_tensor("adjt", [16, 128, 2 * S], FP8, kind="ExternalInput")
    rrs = nc.dram_tensor("rrs", [1, S], BF16, kind="ExternalInput")
    wkv = nc.dram_tensor("wkv", [2, 128, 2 * HD], BF16, kind="ExternalInput")
    bkv = nc.dram_tensor("bkv", [1, 2 * HD], BF16, kind="ExternalInput")
    wqt = nc.dram_tensor("wqt", [2, 128, HD], BF16, kind="ExternalInput")
    bqc = nc.dram_tensor("bqc", [2, 128, 1], F32, kind="ExternalInput")
    i4a_d = nc.dram_tensor("i4a_in", [128, 4], BF16, kind="ExternalInput")
    i4b_d = nc.dram_tensor("i4b_in", [128, 4], BF16, kind="ExternalInput")
    ibcb_d = nc.dram_tensor("ibcb_in", [B, 2, 32, 128], BF16,
                            kind="ExternalInput")
    ones_r_d = nc.dram_tensor("ones_r", [1, S], BF16, kind="ExternalInput")
    ones_c_d = nc.dram_tensor("ones_c", [128, 1], BF16, kind="ExternalInput")
    out = nc.dram_tensor("out", [B, D, S], F32, kind="ExternalOutput")

    with nc.allow_low_precision(reason="bf16/fp8 rounding intentional"), \
            tile.TileContext(nc) as tc:
        with (
            tc.tile_pool(name="pers", bufs=1) as pp,
            tc.tile_pool(name="work", bufs=3) as wk,
            tc.tile_pool(name="dram", bufs=1, space="DRAM") as dp,
        ):
            vm_loc = [dp.tile([128, B * D], FP8, tag=f"vml{j}",
                              name=f"vml{j}") for j in range(4)]
            vm_all = [dp.tile([NCORES, 128, B * D], FP8, tag=f"vma{j}",
                              name=f"vma{j}", addr_space="Shared")
                      for j in range(4)]
            ar_in = dp.tile([2, 130, B, 66], BF16, tag="ar_in", name="ar_in")
            ar_out = dp.tile([2, 130, B, 66], BF16, tag="ar_out",
                             name="ar_out", addr_space="Shared")

            # ---- constants ----
            wkv_t = [pp.tile([128, 2 * HD], BF16, tag=f"wkv{c}",
                             name=f"wkv{c}") for c in range(2)]
            wq_t = [pp.tile([128, HD], BF16, tag=f"wq{c}", name=f"wq{c}")
                    for c in range(2)]
            for c in range(2):
                nc.sync.dma_start(out=wkv_t[c][:], in_=wkv[c])
                nc.sync.dma_start(out=wq_t[c][:], in_=wqt[c])
            bkv_row = pp.tile([1, 2 * HD], BF16, tag="bkvrow")
            nc.sync.dma_start(out=bkv_row[:], in_=bkv[:])
            bq_col = [pp.tile([128, 1], F32, tag=f"bqc{h}", name=f"bqc{h}")
                      for h in range(2)]
            for h in range(2):
                nc.sync.dma_start(out=bq_col[h][:], in_=bqc[h])
            i4 = [pp.tile([128, 4], BF16, tag=f"i4{h}", name=f"i4{h}")
                  for h in range(2)]
            nc.sync.dma_start(out=i4[0][:], in_=i4a_d[:])
            nc.sync.dma_start(out=i4[1][:], in_=i4b_d[:])
            ibcb_sb = [[pp.tile([32, 128], BF16, tag=f"ibcb{b}_{i}",
                                name=f"ibcb{b}_{i}") for i in range(2)]
                       for b in range(B)]
            for b in range(B):
                for i in range(2):
                    nc.sync.dma_start(out=ibcb_sb[b][i][:], in_=ibcb_d[b, i])
            ones_row = pp.tile([1, S], BF16, tag="ones_row")
            ones_col = pp.tile([128, 1], BF16, tag="ones_col")
            nc.sync.dma_start(out=ones_row[:], in_=ones_r_d[:])
            nc.sync.dma_start(out=ones_col[:], in_=ones_c_d[:])
            rrs_row = pp.tile([1, S], BF16, tag="rrs_row")
            nc.sync.dma_start(out=rrs_row[:], in_=rrs[:])

            # xs and xq resident for the whole kernel
            xs_sb = [[pp.tile([128, S], BF16, tag=f"xs{b}_{c}",
                              name=f"xs{b}_{c}") for c in range(2)]
                     for b in range(B)]
            xq_sb = [[pp.tile([128, S], BF16, tag=f"xq{b}_{c}",
                              name=f"xq{b}_{c}") for c in range(2)]
                     for b in range(B)]
            for b in range(B):
                for c in range(2):
                    nc.sync.dma_start(out=xs_sb[b][c][:], in_=xs[b, c])
            # xq/adjT preloads are issued AFTER the collectives (below) so
            # they don't delay the xs tiles feeding the first projections
            adj_sb = [pp.tile([128, 2 * S], FP8, tag=f"adj{m}", name=f"adj{m}")
                      for m in range(16)]

            # persistent SBUF tensors
            vt_sb = [[pp.tile([128, HD + 1], BF16, tag=f"vt{b}_{s}",
                              name=f"vt{b}_{s}") for s in range(4)]
                     for b in range(B)]
            q_sb = [[pp.tile([128, S], BF16, tag=f"q{b}_{h}", name=f"q{b}_{h}")
                     for h in range(2)] for b in range(B)]
            qsq_sb = [[pp.tile([128, S], BF16, tag=f"qq{b}_{h}",
                               name=f"qq{b}_{h}") for h in range(2)]
                      for b in range(B)]
            vm_sb = [pp.tile([128, B * D], FP8, tag=f"vmsb{j}",
                             name=f"vmsb{j}") for j in range(4)]
            sq4N_all = pp.tile([32, S], F32, tag="sq4N", name="sq4N")
            t4_all = pp.tile([32, S], F32, tag="t4", name="t4")
            u4_all = pp.tile([32, S], F32, tag="u4", name="u4")
            w1_bf = pp.tile([32, S], BF16, tag="w1bf", name="w1bf")
            w2_bf = pp.tile([32, S], BF16, tag="w2bf", name="w2bf")
            vsp_all = pp.tile([4, B * D], BF16, tag="vspall", name="vspall")
            attn_sb = [pp.tile([128, S], F32, tag=f"at{p}", name=f"at{p}")
                       for p in range(4)]
            rrs_bc = pp.tile([128, S], F32, tag="rrs_bc")

            # ===== phase VK: concat K|V projection (sb-major) + kvs =====
            with tc.tile_pool(name="psA", bufs=1, space="PSUM") as psA:
                prr = psA.tile([128, S], F32, tag="prr")
                nc.tensor.matmul(prr[:], lhsT=ones_row[:, 0:128],
                                 rhs=rrs_row[:], start=True, stop=True)
                nc.scalar.activation(rrs_bc[:], prr[:], ACTF.Copy)

                kt_hist = {}

                def kv_proj(b, sb):
                    sl = slice(sb * 128, (sb + 1) * 128)
                    pkv = psA.tile([128, 2 * HD], F32, tag="pkv", bufs=3)
                    nc.tensor.matmul(pkv[:], lhsT=xs_sb[b][0][:, sl],
                                     rhs=wkv_t[0][:], start=True, stop=False)
                    nc.tensor.matmul(pkv[:], lhsT=xs_sb[b][1][:, sl],
                                     rhs=wkv_t[1][:], start=False, stop=False)
                    nc.tensor.matmul(pkv[:], lhsT=ones_row[:, 0:128],
                                     rhs=bkv_row[:], start=False, stop=True)
                    vt = vt_sb[b][sb]
                    nc.scalar.activation(vt[:, 0:HD], pkv[:, HD:2 * HD],
                                         ACTF.Copy)
                    nc.vector.memset(vt[:, HD:HD + 1], 1.0)
                    nc.vector.reduce_sum(
                        vm_sb[sb][:, b * D:(b + 1) * D],
                        pkv[:, HD:2 * HD].rearrange("p (h d) -> p d h", h=H),
                        axis=AX)
                    ksq = wk.tile([128, HD], F32, tag="ksq", bufs=2)
                    nc.scalar.activation(ksq[:], pkv[:, 0:HD], ACTF.Square)
                    ssk = wk.tile([128, H], F32, tag="ssk", bufs=2)
                    nc.vector.reduce_sum(
                        ssk[:], ksq[:].rearrange("p (h d) -> p h d", h=H),
                        axis=AX)
                    snk = wk.tile([128, H], F32, tag="snk", bufs=2)
                    nc.scalar.activation(snk[:], ssk[:], ACTF.Sqrt)
                    rk = wk.tile([128, H], F32, tag="rk", bufs=2)
                    nc.vector.reciprocal(rk[:], snk[:])
                    kt = wk.tile([128, HD], BF16, tag=f"kt{b}_{sb}",
                                 name=f"kt{b}_{sb}", bufs=1)
                    nc.vector.tensor_mul(
                        kt[:].rearrange("p (h d) -> p h d", h=H),
                        pkv[:, 0:HD].rearrange("p (h d) -> p h d", h=H),
                        rk[:].broadcast_to((128, H, D)))
                    kt_hist.setdefault(b, []).append(kt)

                def kvs_phase(b):
                    kt_t = kt_hist.pop(b)
                    kvs0 = psA.tile([128, HD + 1], F32, tag="kvs0", bufs=1)
                    kvs1 = psA.tile([128, HD + 1], F32, tag="kvs1", bufs=1)
                    vs_ps = psA.tile([1, HD + 1], F32, tag="vsps", bufs=1)
                    for sb in range(4):
                        nc.tensor.matmul(kvs0[:], lhsT=kt_t[sb][:, 0:128],
                                         rhs=vt_sb[b][sb][:],
                                         start=(sb == 0), stop=(sb == 3))
                    for sb in range(4):
                        nc.tensor.matmul(kvs1[:], lhsT=kt_t[sb][:, 128:HD],
                                         rhs=vt_sb[b][sb][:],
                                         start=(sb == 0), stop=(sb == 3))
                    for sb in range(4):
                        nc.tensor.matmul(vs_ps[:], lhsT=ones_col[:],
                                         rhs=vt_sb[b][sb][:],
                                         start=(sb == 0), stop=(sb == 3))
                    for i, kvs in ((0, kvs0), (1, kvs1)):
                        pkx = wk.tile([128, 66], BF16, tag=f"pkx{i}", bufs=2)
                        nc.scalar.activation(pkx[0:64, 0:64],
                                             kvs[0:64, 128 * i:128 * i + 64],
                                             ACTF.Copy)
                        nc.scalar.activation(
                            pkx[64:128, 0:64],
                            kvs[64:128, 128 * i + 64:128 * i + 128], ACTF.Copy)
                        nc.vector.memset(pkx[:, 64:66], 0.0)
                        # ks columns scaled by 4 (folds the 4*t of u4)
                        nc.scalar.activation(pkx[0:64, 64:65],
                                             kvs[0:64, HD:HD + 1], ACTF.Copy,
                                             scale=4.0)
                        nc.scalar.activation(pkx[64:128, 65:66],
                                             kvs[64:128, HD:HD + 1], ACTF.Copy,
                                             scale=4.0)
                        nc.sync.dma_start(out=ar_in[i, 0:128, b, :],
                                          in_=pkx[:])
                    ksvs = wk.tile([1, 264], BF16, tag="ksvs", bufs=2)
                    nc.vector.memset(ksvs[:], 0.0)
                    for h in range(H):
                        nc.scalar.activation(
                            ksvs[0:1, 66 * h:66 * h + 64],
                            vs_ps[0:1, 64 * h:64 * h + 64], ACTF.Copy)
                    nc.sync.dma_start(out=ar_in[0, 128:130, b, :],
                                      in_=ksvs[0:1, 0:132])
                    nc.sync.dma_start(out=ar_in[1, 128:130, b, :],
                                      in_=ksvs[0:1, 132:264])

                ag_launched = 0
                for sb in range(4):
                    for b in range(B):
                        kv_proj(b, sb)
                        if sb == 3 and b > 0:
                            kvs_phase(b - 1)
                    # chunk sb complete: one fat DMA out, then its AllGather
                    nc.sync.dma_start(out=vm_loc[sb][:], in_=vm_sb[sb][:])
                    nc.gpsimd.collective_compute(
                        "AllGather", ALU.bypass, ins=[vm_loc[sb].opt()],
                        outs=[vm_all[sb].opt()], replica_groups=RG)
                kvs_phase(B - 1)

            nc.gpsimd.collective_compute(
                "AllReduce", ALU.add, ins=[ar_in.opt()],
                outs=[ar_out.opt()], replica_groups=RG)

            # bulk preloads for the later phases (issued late so they don't
            # delay the xs tiles feeding the first projections)
            for b in range(B):
                for c in range(2):
                    nc.sync.dma_start(out=xq_sb[b][c][:], in_=xq[b, c])
            for m in range(16):
                nc.sync.dma_start(out=adj_sb[m][:], in_=adjt[m])

            # ============ phase Q (weight-stationary) ======================
            with tc.tile_pool(name="psC", bufs=1, space="PSUM") as psC:
                for half in range(2):
                    hsl = slice(half * 128, (half + 1) * 128)
                    for bb in (0, 4):
                        pqs = [psC.tile([128, S], F32, tag=f"pq{i}",
                                        name=f"pq{i}", bufs=1)
                               for i in range(4)]
                        for c in range(2):
                            for i in range(4):
                                nc.tensor.matmul(
                                    pqs[i][:], lhsT=wq_t[c][:, hsl],
                                    rhs=xq_sb[bb + i][c][:],
                                    start=(c == 0), stop=(c == 1))
                        for i in range(4):
                            b = bb + i
                            nc.scalar.activation(q_sb[b][half][:], pqs[i][:],
                                                 ACTF.Identity,
                                                 bias=bq_col[half][:])
                            nc.vector.tensor_mul(qsq_sb[b][half][:],
                                                 q_sb[b][half][:],
                                                 q_sb[b][half][:])
                for b in range(B):
                    ssp = psC.tile([4, S], F32, tag="ss", bufs=2)
                    nc.tensor.matmul(ssp[:], lhsT=i4[0][:],
                                     rhs=qsq_sb[b][0][:], start=True,
                                     stop=False)
                    nc.tensor.matmul(ssp[:], lhsT=i4[1][:],
                                     rhs=qsq_sb[b][1][:], start=False,
                                     stop=True)
                    sqb = wk.tile([4, S], F32, tag="sqb", bufs=2)
                    nc.scalar.activation(sqb[:], ssp[:],
                                         ACTF.Sqrt, scale=float(16 * N * N))
                    nc.sync.dma_start(out=sq4N_all[4 * b:4 * b + 4, :],
                                      in_=sqb[:])

            # ============ GCN (DoubleRow fp8) + attention epilogue =========
            with tc.tile_pool(name="psD", bufs=1, space="PSUM") as psD:
                pg = [psD.tile([128, S], F32, tag=f"g{p}", name=f"g{p}")
                      for p in range(4)]
                if USE_DOUBLE_ROW:
                    for jp in range(2):
                        for peer in range(NCORES):
                            vmt = wk.tile([128, 2 * B * D], FP8, tag="vml",
                                          bufs=3)
                            nc.sync.dma_start(out=vmt[:, 0:512],
                                              in_=vm_all[2 * jp][peer])
                            nc.sync.dma_start(out=vmt[:, 512:1024],
                                              in_=vm_all[2 * jp + 1][peer])
                            vmv = vmt[:].rearrange("p (o f) -> p o f", o=2)
                            adv = adj_sb[peer * 2 + jp][:].rearrange(
                                "p (o f) -> p o f", o=2)
                            for p in range(4):
                                nc.tensor.matmul(
                                    pg[p][:],
                                    lhsT=vmv[:, :, 128 * p:128 * (p + 1)],
                                    rhs=adv[:],
                                    start=(jp == 0 and peer == 0),
                                    stop=(jp == 1 and peer == 7),
                                    perf_mode=PERF.DoubleRow)
                else:
                    for j in range(4):
                        for peer in range(NCORES):
                            vmt = wk.tile([128, B * D], FP8, tag="vml", bufs=3)
                            nc.sync.dma_start(out=vmt[:], in_=vm_all[j][peer])
                            adv = adj_sb[peer * 2 + j // 2][:].rearrange(
                                "p (o f) -> p o f", o=2)
                            for p in range(4):
                                nc.tensor.matmul(
                                    pg[p][:],
                                    lhsT=vmt[:, 128 * p:128 * (p + 1)],
                                    rhs=adv[:, j % 2, :],
                                    start=(j == 0 and peer == 0),
                                    stop=(j == 3 and peer == 7))

                # ---- attention epilogue (AllReduce landed during GCN) ----
                kpx_all = [pp.tile([128, B * 66], BF16, tag=f"kpxa{i}",
                                   name=f"kpxa{i}") for i in range(2)]
                for i in range(2):
                    nc.sync.dma_start(out=kpx_all[i][:],
                                      in_=ar_out[i, 0:128, :, :])
                for b in range(B):
                    for i in range(2):
                        pden = psD.tile([2, S], F32, tag="pden", bufs=2)
                        nc.tensor.matmul(
                            pden[:],
                            lhsT=kpx_all[i][:, 66 * b + 64:66 * b + 66],
                            rhs=q_sb[b][i][:], start=True, stop=True)
                        tt = wk.tile([2, S], F32, tag="tt", bufs=2)
                        nc.scalar.activation(tt[:], pden[:], ACTF.Copy)
                        nc.sync.dma_start(
                            out=t4_all[4 * b + 2 * i:4 * b + 2 * i + 2, :],
                            in_=tt[:])
                vspf = wk.tile([4, B * D], BF16, tag="vspf", bufs=1)
                for i in range(2):
                    nc.sync.dma_start(out=vspf[2 * i:2 * i + 2, :],
                                      in_=ar_out[i, 128:130, :, 0:64])
                nc.scalar.activation(vsp_all[:], vspf[:], ACTF.Copy,
                                     scale=float(1.0 / (4 * N)))
                nc.vector.tensor_add(u4_all[:], t4_all[:], sq4N_all[:])
                w1f = wk.tile([32, S], F32, tag="w1f", bufs=1)
                nc.vector.reciprocal_approx_fast(w1f[:], u4_all[:])
                nc.scalar.activation(w1_bf[:], w1f[:], ACTF.Copy)
                nc.vector.tensor_mul(w2_bf[:], w1f[:], sq4N_all[:])

                for b in range(B):
                    w2b = wk.tile([4, S], BF16, tag="w2b", bufs=2)
                    nc.sync.dma_start(out=w2b[:],
                                      in_=w2_bf[4 * b:4 * b + 4, :])
                    qs_t = []
                    for i in range(2):
                        pbc = psD.tile([128, S], F32, tag="pbc", bufs=1)
                        nc.tensor.matmul(pbc[:], lhsT=ibcb_sb[b][i][:],
                                         rhs=w1_bf[:], start=True, stop=True)
                        qs = wk.tile([128, S], BF16, tag=f"qs{i}", bufs=2)
                        nc.vector.tensor_mul(qs[:], q_sb[b][i][:], pbc[:])
                        qs_t.append(qs)
                    pat = psD.tile([D, S], F32, tag="pat", bufs=1)
                    nc.tensor.matmul(pat[:],
                                     lhsT=kpx_all[0][:, 66 * b:66 * b + 64],
                                     rhs=qs_t[0][:], start=True, stop=False)
                    nc.tensor.matmul(pat[:],
                                     lhsT=kpx_all[1][:, 66 * b:66 * b + 64],
                                     rhs=qs_t[1][:], start=False, stop=False)
                    nc.tensor.matmul(pat[:],
                                     lhsT=vsp_all[:, b * D:(b + 1) * D],
                                     rhs=w2b[:], start=False, stop=True)
                    nc.scalar.activation(
                        attn_sb[b // 2][(b % 2) * D:(b % 2 + 1) * D, :],
                        pat[:], ACTF.Copy)

                for p in range(4):
                    gt = wk.tile([128, S], F32, tag="gt", bufs=2)
                    nc.vector.tensor_mul(gt[:], pg[p][:], rrs_bc[:])
                    ot = wk.tile([128, S], F32, tag="ot", bufs=2)
                    nc.vector.tensor_add(ot[:], gt[:], attn_sb[p][:])
                    nc.sync.dma_start(out=out[2 * p], in_=ot[0:D, :])
                    nc.sync.dma_start(out=out[2 * p + 1], in_=ot[D:128, :])
    nc.compile()
    return nc


def _prep_inputs(query_input, source_input, adj, Wq_w, Wq_b, Wk_w, Wk_b,
                 Wv_w, Wv_b):
    bf16 = ml_dtypes.bfloat16
    fp8 = ml_dtypes.float8_e4m3fn
    xq_np = np.asarray(query_input, dtype=np.float32)
    xs_np = np.asarray(source_input, dtype=np.float32)
    adj_np = np.asarray(adj, dtype=np.float32)

    adjT = np.ascontiguousarray(adj_np.T)
    np.fill_diagonal(adjT, adjT.diagonal() + 1.0)
    adjT_f8 = adjT.astype(fp8)
    rrs_full = (0.25 / (adj_np.sum(axis=1) + 1.0)).astype(bf16)

    wkv_np = np.concatenate([np.asarray(Wk_w, np.float32).T,
                             np.asarray(Wv_w, np.float32).T], axis=1)
    wkv_np = np.ascontiguousarray(wkv_np).astype(bf16).reshape(2, 128, 2 * HD)
    bkv_np = np.concatenate([np.asarray(Wk_b, np.float32),
                             np.asarray(Wv_b, np.float32)])
    bkv_np = bkv_np.astype(bf16).reshape(1, 2 * HD)
    wqt = np.ascontiguousarray(np.asarray(Wq_w, np.float32).T)
    wqt = wqt.astype(bf16).reshape(2, 128, HD)
    bqc = np.asarray(Wq_b, np.float32).reshape(2, 128, 1)

    i4a, i4b, ibcb = _indicators()
    in_maps = []
    for i in range(NCORES):
        sl = slice(i * S, (i + 1) * S)
        # [4096, S] -> [peer, jp, o, ki, n] -> [peer, jp, ki, o, n]
        a = adjT_f8[:, sl].reshape(8, 2, 2, 128, S)
        a = np.ascontiguousarray(a.transpose(0, 1, 3, 2, 4))
        in_maps.append({
            "xq": np.ascontiguousarray(xq_np[:, :, sl]).astype(bf16)
                  .reshape(B, 2, 128, S),
            "xs": np.ascontiguousarray(xs_np[:, :, sl]).astype(bf16)
                  .reshape(B, 2, 128, S),
            "adjt": a.reshape(16, 128, 2 * S),
            "rrs": np.ascontiguousarray(rrs_full[sl]).reshape(1, S),
            "wkv": wkv_np, "bkv": bkv_np, "wqt": wqt, "bqc": bqc,
            "i4a_in": i4a.astype(bf16), "i4b_in": i4b.astype(bf16),
            "ibcb_in": ibcb.astype(bf16),
            "ones_r": np.ones((1, S), bf16),
            "ones_c": np.ones((128, 1), bf16),
        })
    return in_maps


def kernel(**inputs):
    if "nc" not in _CACHE:
        _CACHE["nc"] = _build()
    nc = _CACHE["nc"]
    in_maps = _prep_inputs(**inputs)
    res = run_bass_kernel_spmd(nc, in_maps, list(range(NCORES)))
    full = np.empty((B, D, N), np.float32)
    for i in range(NCORES):
        full[:, :, i * S:(i + 1) * S] = res.results[i]["out"]
    return full


# revision 35
# speedup vs baseline: 2.5246x; 1.0722x over previous
"""DIFFormerConv (simple linear attention + dense GCN) on 8 trn2 NeuronCores.

Sharding: nodes N=4096 split 8 ways (S=512 per core). Phase order is chosen
so both collectives hide under compute:
  V-pass (vmean -> fp8)  -> 4 chunked AllGathers start ~15% in
  K-pass + kvs partials  -> bf16 AllReduce (kvs diag blocks | ks cols | vs)
  Q-pass (weight-stationary, bf16)
  GCN (DoubleRow fp8: adjT resident in SBUF, vmean pairs as lhsT)
  attention epilogue (needs AllReduce, which landed during GCN)
  combine + store

Layouts (no PE transposes anywhere):
  q:   [hd, s]   (heads*dim on partitions)  -- lhsT = Wq^T chunks (stationary)
  k,v: [s, hd]   (transposed projection)    -- lhsT = xs chunks (stationary)
  kvs: lhsT = kt chunk, rhs = [vt | ones]   -- ks falls out as PSUM column 256
  gcn: [(b,d), n] -- lhsT = vm pairs [128,2,128] fp8, rhs = adjT [128,2,512]
Denominator algebra (one stacked [32,S] approx reciprocal):
  w1 = 1/(4*t + 4*N*sqrt(ss)),  w2 = 4*N*sqrt(ss)*w1,  t = q . ks (raw q)
  attn = sum_h kvs_h^T @ (q_h * w1_h) + (vs/(4N))^T @ w2
The PE on this part runs at 1.2 GHz regardless of HAM, so MM cycles are
minimized (DoubleRow fp8 GCN, N>=256 everywhere, dense issue order).
"""

import sys

sys.path.insert(0, "/opt/trn_rl_repo")

import numpy as np
import ml_dtypes

from concourse import bass, bacc, tile, mybir
from concourse.bass_utils import run_bass_kernel_spmd

B, C, N, H, D = 8, 256, 4096, 4, 64
NCORES = 8
S = N // NCORES          # 512 nodes per core
HD = H * D               # 256
F32 = mybir.dt.float32
BF16 = mybir.dt.bfloat16
FP8 = mybir.dt.float8e4
AX = mybir.AxisListType.X
ALU = mybir.AluOpType
ACTF = mybir.ActivationFunctionType
PERF = mybir.MatmulPerfMode
RG = [list(range(NCORES))]

USE_DOUBLE_ROW = True

_CACHE = {}


def _indicators():
    i4o = np.zeros((128, 9), np.float32)
    for p in range(128):
        i4o[p, p // 64] = 1.0       # i4a cols 0:4
        i4o[p, 6 + p // 64] = 1.0   # i4b cols 4:8 -> 2 + p//64 within
        i4o[p, 8] = 1.0             # ones column
    ibcb = np.zeros((B, 2, 32, 128), np.float32)
    for b in range(B):
        for i in range(2):
            for p in range(128):
                ibcb[b, i, 4 * b + 2 * i + p // 64, p] = 1.0
    ibcb = ibcb.transpose(2, 0, 1, 3).reshape(32, B * 2 * 128)
    return i4o, ibcb


def _build():
    nc = bacc.Bacc("TRN2", target_bir_lowering=False, debug=False,
                   num_devices=NCORES)

    xq = nc.dram_tensor("xq", [B, 2, 128, S], BF16, kind="ExternalInput")
    xs = nc.dram_tensor("xs", [B, 2, 128, S], BF16, kind="ExternalInput")
    # DoubleRow layout: [peer*2+jp, ki, o, n] = adjT[peer*512+jp*256+o*128+ki, n]
    adjt = nc.dram_tensor("adjt", [16, 128, 2 * S], FP8, kind="ExternalInput")
    wkv = nc.dram_tensor("wkv", [2, 128, 2 * HD], BF16, kind="ExternalInput")
    bkv = nc.dram_tensor("bkv", [1, 2 * HD], BF16, kind="ExternalInput")
    wqt = nc.dram_tensor("wqt", [2, 128, HD], BF16, kind="ExternalInput")
    bqc = nc.dram_tensor("bqc", [2, 128, 1], F32, kind="ExternalInput")
    i4o_d = nc.dram_tensor("i4o_in", [128, 9], BF16, kind="ExternalInput")
    ibcb_d = nc.dram_tensor("ibcb_in", [32, B * 2 * 128], BF16,
                            kind="ExternalInput")
    orr_d = nc.dram_tensor("orr_in", [1, 2 * S], BF16, kind="ExternalInput")
    out = nc.dram_tensor("out", [B, D, S], F32, kind="ExternalOutput")

    with nc.allow_low_precision(reason="bf16/fp8 rounding intentional"), \
            tile.TileContext(nc) as tc:
        with (
            tc.tile_pool(name="pers", bufs=1) as pp,
            tc.tile_pool(name="work", bufs=3) as wk,
            tc.tile_pool(name="dram", bufs=1, space="DRAM") as dp,
        ):
            vm_loc = [dp.tile([128, B * D], FP8, tag=f"vml{j}",
                              name=f"vml{j}") for j in range(4)]
            vm_all = [dp.tile([NCORES, 128, B * D], FP8, tag=f"vma{j}",
                              name=f"vma{j}", addr_space="Shared")
                      for j in range(4)]
            ar_in = dp.tile([2, 130, B, 66], BF16, tag="ar_in", name="ar_in")
            ar_out = dp.tile([2, 130, B, 66], BF16, tag="ar_out",
                             name="ar_out", addr_space="Shared")

            # ---- constants (batched into few fat DMAs) ----
            wkv_t = [pp.tile([128, 2 * HD], BF16, tag=f"wkv{c}",
                             name=f"wkv{c}") for c in range(2)]
            wq_t = [pp.tile([128, HD], BF16, tag=f"wq{c}", name=f"wq{c}")
                    for c in range(2)]
            orr = pp.tile([1, 2 * S], BF16, tag="orr")
            bkv_row = pp.tile([1, 2 * HD], BF16, tag="bkvrow")
            for c in range(2):
                nc.sync.dma_start(out=wkv_t[c][:], in_=wkv[c])
            nc.sync.dma_start(out=orr[:], in_=orr_d[:])
            nc.sync.dma_start(out=bkv_row[:], in_=bkv[:])
            i4o = pp.tile([128, 9], BF16, tag="i4o")
            nc.sync.dma_start(out=i4o[:], in_=i4o_d[:])
            bq_col = [pp.tile([128, 1], F32, tag=f"bqc{h}", name=f"bqc{h}")
                      for h in range(2)]
            for h in range(2):
                nc.sync.dma_start(out=bq_col[h][:], in_=bqc[h])
            for c in range(2):
                nc.sync.dma_start(out=wq_t[c][:], in_=wqt[c])
            ibcb_all = pp.tile([32, B * 2 * 128], BF16, tag="ibcball")
            nc.sync.dma_start(out=ibcb_all[:], in_=ibcb_d[:])

            # xs and xq resident for the whole kernel
            xs_sb = [[pp.tile([128, S], BF16, tag=f"xs{b}_{c}",
                              name=f"xs{b}_{c}") for c in range(2)]
                     for b in range(B)]
            xq_sb = [[pp.tile([128, S], BF16, tag=f"xq{b}_{c}",
                              name=f"xq{b}_{c}") for c in range(2)]
                     for b in range(B)]
            for b in range(B):
                for c in range(2):
                    nc.sync.dma_start(out=xs_sb[b][c][:], in_=xs[b, c])
            for b in range(B):
                for c in range(2):
                    nc.sync.dma_start(out=xq_sb[b][c][:], in_=xq[b, c])
            # adjT preloads are issued after the collectives so they don't
            # delay the xs/xq tiles feeding the projections
            adj_sb = [pp.tile([128, 2 * S], FP8, tag=f"adj{m}", name=f"adj{m}")
                      for m in range(16)]

            # persistent SBUF tensors
            vt_sb = [[pp.tile([128, HD + 1], BF16, tag=f"vt{b}_{s}",
                              name=f"vt{b}_{s}") for s in range(4)]
                     for b in range(B)]
            q_sb = [[pp.tile([128, S], BF16, tag=f"q{b}_{h}", name=f"q{b}_{h}")
                     for h in range(2)] for b in range(B)]
            qsq_sb = [[pp.tile([128, S], BF16, tag=f"qq{b}_{h}",
                               name=f"qq{b}_{h}") for h in range(2)]
                      for b in range(B)]
            vm_sb = [pp.tile([128, B * D], FP8, tag=f"vmsb{j}",
                             name=f"vmsb{j}") for j in range(4)]
            sq4N_all = pp.tile([32, S], F32, tag="sq4N", name="sq4N")
            t4_all = pp.tile([32, S], F32, tag="t4", name="t4")
            u4_all = pp.tile([32, S], F32, tag="u4", name="u4")
            w1_bf = pp.tile([32, S], BF16, tag="w1bf", name="w1bf")
            w2_bf = pp.tile([32, S], BF16, tag="w2bf", name="w2bf")
            vsp_all = pp.tile([4, B * D], BF16, tag="vspall", name="vspall")
            attn_sb = [pp.tile([128, S], F32, tag=f"at{p}", name=f"at{p}")
                       for p in range(4)]
            rrs_bc = pp.tile([128, S], F32, tag="rrs_bc")

            # ===== phase VK: concat K|V projection (sb-major) + kvs =====
            with tc.tile_pool(name="psA", bufs=1, space="PSUM") as psA:
                kt_hist = {}

                def kv_proj(b, sb):
                    sl = slice(sb * 128, (sb + 1) * 128)
                    pkv = psA.tile([128, 2 * HD], F32, tag="pkv", bufs=2)
                    nc.tensor.matmul(pkv[:], lhsT=xs_sb[b][0][:, sl],
                                     rhs=wkv_t[0][:], start=True, stop=False)
                    nc.tensor.matmul(pkv[:], lhsT=xs_sb[b][1][:, sl],
                                     rhs=wkv_t[1][:], start=False, stop=False)
                    nc.tensor.matmul(pkv[:], lhsT=orr[0:1, 0:128],
                                     rhs=bkv_row[:], start=False, stop=True)
                    vt = vt_sb[b][sb]
                    nc.scalar.activation(vt[:, 0:HD], pkv[:, HD:2 * HD],
                                         ACTF.Copy)
                    nc.vector.memset(vt[:, HD:HD + 1], 1.0)
                    nc.vector.reduce_sum(
                        vm_sb[sb][:, b * D:(b + 1) * D],
                        pkv[:, HD:2 * HD].rearrange("p (h d) -> p d h", h=H),
                        axis=AX)
                    ksq = wk.tile([128, HD], F32, tag="ksq", bufs=2)
                    nc.scalar.activation(ksq[:], pkv[:, 0:HD], ACTF.Square)
                    ssk = wk.tile([128, H], F32, tag="ssk", bufs=2)
                    nc.vector.reduce_sum(
                        ssk[:], ksq[:].rearrange("p (h d) -> p h d", h=H),
                        axis=AX)
                    snk = wk.tile([128, H], F32, tag="snk", bufs=2)
                    nc.scalar.activation(snk[:], ssk[:], ACTF.Sqrt)
                    rk = wk.tile([128, H], F32, tag="rk", bufs=2)
                    nc.vector.reciprocal(rk[:], snk[:])
                    kt = wk.tile([128, HD], BF16, tag=f"kt{b}_{sb}",
                                 name=f"kt{b}_{sb}", bufs=1)
                    nc.vector.tensor_mul(
                        kt[:].rearrange("p (h d) -> p h d", h=H),
                        pkv[:, 0:HD].rearrange("p (h d) -> p h d", h=H),
                        rk[:].broadcast_to((128, H, D)))
                    kt_hist.setdefault(b, []).append(kt)

                def kvs_phase(b):
                    kt_t = kt_hist.pop(b)
                    kvs0 = psA.tile([128, HD + 1], F32, tag="kvs0", bufs=1)
                    kvs1 = psA.tile([128, HD + 1], F32, tag="kvs1", bufs=1)
                    vs_ps = psA.tile([1, HD + 1], F32, tag="vsps", bufs=1)
                    for sb in range(4):
                        nc.tensor.matmul(kvs0[:], lhsT=kt_t[sb][:, 0:128],
                                         rhs=vt_sb[b][sb][:],
                                         start=(sb == 0), stop=(sb == 3))
                    for sb in range(4):
                        nc.tensor.matmul(kvs1[:], lhsT=kt_t[sb][:, 128:HD],
                                         rhs=vt_sb[b][sb][:],
                                         start=(sb == 0), stop=(sb == 3))
                    for sb in range(4):
                        nc.tensor.matmul(vs_ps[:], lhsT=i4o[:, 8:9],
                                         rhs=vt_sb[b][sb][:],
                                         start=(sb == 0), stop=(sb == 3))
                    for i, kvs in ((0, kvs0), (1, kvs1)):
                        pkx = wk.tile([128, 66], BF16, tag=f"pkx{i}", bufs=2)
                        nc.scalar.activation(pkx[0:64, 0:64],
                                             kvs[0:64, 128 * i:128 * i + 64],
                                             ACTF.Copy)
                        nc.scalar.activation(
                            pkx[64:128, 0:64],
                            kvs[64:128, 128 * i + 64:128 * i + 128], ACTF.Copy)
                        nc.vector.memset(pkx[:, 64:66], 0.0)
                        # ks columns scaled by 4 (folds the 4*t of u4)
                        nc.scalar.activation(pkx[0:64, 64:65],
                                             kvs[0:64, HD:HD + 1], ACTF.Copy,
                                             scale=4.0)
                        nc.scalar.activation(pkx[64:128, 65:66],
                                             kvs[64:128, HD:HD + 1], ACTF.Copy,
                                             scale=4.0)
                        nc.sync.dma_start(out=ar_in[i, 0:128, b, :],
                                          in_=pkx[:])
                    ksvs = wk.tile([1, 264], BF16, tag="ksvs", bufs=2)
                    nc.vector.memset(ksvs[:], 0.0)
                    for h in range(H):
                        nc.scalar.activation(
                            ksvs[0:1, 66 * h:66 * h + 64],
                            vs_ps[0:1, 64 * h:64 * h + 64], ACTF.Copy)
                    nc.sync.dma_start(out=ar_in[0, 128:130, b, :],
                                      in_=ksvs[0:1, 0:132])
                    nc.sync.dma_start(out=ar_in[1, 128:130, b, :],
                                      in_=ksvs[0:1, 132:264])

                def q_block(j):
                    half, base = j // 2, (j % 2) * 4
                    hsl = slice(half * 128, (half + 1) * 128)
                    for bb in (base, base + 2):
                        pqs = [psA.tile([128, S], F32, tag=f"pq{i}",
                                        name=f"pq{i}", bufs=1)
                               for i in range(2)]
                        for c in range(2):
                            for i in range(2):
                                nc.tensor.matmul(
                                    pqs[i][:], lhsT=wq_t[c][:, hsl],
                                    rhs=xq_sb[bb + i][c][:],
                                    start=(c == 0), stop=(c == 1))
                        for i in range(2):
                            b = bb + i
                            nc.scalar.activation(q_sb[b][half][:], pqs[i][:],
                                                 ACTF.Identity,
                                                 bias=bq_col[half][:])
                            nc.vector.tensor_mul(qsq_sb[b][half][:],
                                                 q_sb[b][half][:],
                                                 q_sb[b][half][:])

                def ss_block(b):
                    ssp = psA.tile([4, S], F32, tag="ss", bufs=1)
                    nc.tensor.matmul(ssp[:], lhsT=i4o[:, 0:4],
                                     rhs=qsq_sb[b][0][:], start=True,
                                     stop=False)
                    nc.tensor.matmul(ssp[:], lhsT=i4o[:, 4:8],
                                     rhs=qsq_sb[b][1][:], start=False,
                                     stop=True)
                    sqb = wk.tile([4, S], F32, tag="sqb", bufs=2)
                    nc.scalar.activation(sqb[:], ssp[:],
                                         ACTF.Sqrt, scale=float(16 * N * N))
                    nc.sync.dma_start(out=sq4N_all[4 * b:4 * b + 4, :],
                                      in_=sqb[:])

                # Q-projection blocks are interleaved between VK chunks so
                # the PE fills the gaps left by the k-norm chains
                for sb in range(4):
                    for b in range(B):
                        kv_proj(b, sb)
                        if sb == 3 and b > 0:
                            kvs_phase(b - 1)
                    # chunk sb complete: one fat DMA out, then its AllGather
                    nc.sync.dma_start(out=vm_loc[sb][:], in_=vm_sb[sb][:])
                    nc.gpsimd.collective_compute(
                        "AllGather", ALU.bypass, ins=[vm_loc[sb].opt()],
                        outs=[vm_all[sb].opt()], replica_groups=RG)
                    q_block(sb)
                    if sb == 2:
                        for b in range(4):
                            ss_block(b)
                kvs_phase(B - 1)
                for b in range(4, 8):
                    ss_block(b)

            nc.gpsimd.collective_compute(
                "AllReduce", ALU.add, ins=[ar_in.opt()],
                outs=[ar_out.opt()], replica_groups=RG)

            for m in range(16):
                nc.sync.dma_start(out=adj_sb[m][:], in_=adjt[m])

            # ============ GCN (DoubleRow fp8) + attention epilogue =========
            with tc.tile_pool(name="psD", bufs=1, space="PSUM") as psD:
                prr = psD.tile([128, S], F32, tag="pbc", name="prr")
                nc.tensor.matmul(prr[:], lhsT=orr[0:1, 0:128],
                                 rhs=orr[0:1, S:2 * S], start=True, stop=True)
                nc.scalar.activation(rrs_bc[:], prr[:], ACTF.Copy)
                pg = [psD.tile([128, S], F32, tag=f"g{p}", name=f"g{p}")
                      for p in range(4)]
                if USE_DOUBLE_ROW:
                    for jp in range(2):
                        for peer in range(NCORES):
                            vmt = wk.tile([128, 2 * B * D], FP8, tag="vml",
                                          bufs=3)
                            nc.sync.dma_start(out=vmt[:, 0:512],
                                              in_=vm_all[2 * jp][peer])
                            nc.sync.dma_start(out=vmt[:, 512:1024],
                                              in_=vm_all[2 * jp + 1][peer])
                            vmv = vmt[:].rearrange("p (o f) -> p o f", o=2)
                            adv = adj_sb[peer * 2 + jp][:].rearrange(
                                "p (o f) -> p o f", o=2)
                            for p in range(4):
                                nc.tensor.matmul(
                                    pg[p][:],
                                    lhsT=vmv[:, :, 128 * p:128 * (p + 1)],
                                    rhs=adv[:],
                                    start=(jp == 0 and peer == 0),
                                    stop=(jp == 1 and peer == 7),
                                    perf_mode=PERF.DoubleRow)
                else:
                    for j in range(4):
                        for peer in range(NCORES):
                            vmt = wk.tile([128, B * D], FP8, tag="vml", bufs=3)
                            nc.sync.dma_start(out=vmt[:], in_=vm_all[j][peer])
                            adv = adj_sb[peer * 2 + j // 2][:].rearrange(
                                "p (o f) -> p o f", o=2)
                            for p in range(4):
                                nc.tensor.matmul(
                                    pg[p][:],
                                    lhsT=vmt[:, 128 * p:128 * (p + 1)],
                                    rhs=adv[:, j % 2, :],
                                    start=(j == 0 and peer == 0),
                                    stop=(j == 3 and peer == 7))

                # ---- attention epilogue (AllReduce landed during GCN) ----
                kpx_all = [pp.tile([128, B * 66], BF16, tag=f"kpxa{i}",
                                   name=f"kpxa{i}") for i in range(2)]
                for i in range(2):
                    nc.sync.dma_start(out=kpx_all[i][:],
                                      in_=ar_out[i, 0:128, :, :])
                for b in range(B):
                    for i in range(2):
                        pden = psD.tile([2, S], F32, tag="pden", bufs=2)
                        nc.tensor.matmul(
                            pden[:],
                            lhsT=kpx_all[i][:, 66 * b + 64:66 * b + 66],
                            rhs=q_sb[b][i][:], start=True, stop=True)
                        tt = wk.tile([2, S], F32, tag="tt", bufs=2)
                        nc.scalar.activation(tt[:], pden[:], ACTF.Copy)
                        nc.sync.dma_start(
                            out=t4_all[4 * b + 2 * i:4 * b + 2 * i + 2, :],
                            in_=tt[:])
                vspf = wk.tile([4, B * D], BF16, tag="vspf", bufs=1)
                for i in range(2):
                    nc.sync.dma_start(out=vspf[2 * i:2 * i + 2, :],
                                      in_=ar_out[i, 128:130, :, 0:64])
                nc.scalar.activation(vsp_all[:], vspf[:], ACTF.Copy,
                                     scale=float(1.0 / (4 * N)))
                nc.vector.tensor_add(u4_all[:], t4_all[:], sq4N_all[:])
                w1f = wk.tile([32, S], F32, tag="w1f", bufs=1)
                nc.vector.reciprocal_approx_fast(w1f[:], u4_all[:])
                nc.scalar.activation(w1_bf[:], w1f[:], ACTF.Copy)
                nc.vector.tensor_mul(w2_bf[:], w1f[:], sq4N_all[:])

                for b in range(B):
                    w2b = wk.tile([4, S], BF16, tag="w2b", bufs=2)
                    nc.sync.dma_start(out=w2b[:],
                                      in_=w2_bf[4 * b:4 * b + 4, :])
                    qs_t = []
                    for i in range(2):
                        pbc = psD.tile([128, S], F32, tag="pbc", bufs=1)
                        nc.tensor.matmul(pbc[:], lhsT=ibcb_all[:, (2 * b + i) * 128:(2 * b + i + 1) * 128],
                                         rhs=w1_bf[:], start=True, stop=True)
                        qs = wk.tile([128, S], BF16, tag=f"qs{i}", bufs=2)
                        nc.vector.tensor_mul(qs[:], q_sb[b][i][:], pbc[:])
                        qs_t.append(qs)
                    pat = psD.tile([D, S], F32, tag="pat", bufs=1)
                    nc.tensor.matmul(pat[:],
                                     lhsT=kpx_all[0][:, 66 * b:66 * b + 64],
                                     rhs=qs_t[0][:], start=True, stop=False)
                    nc.tensor.matmul(pat[:],
                                     lhsT=kpx_all[1][:, 66 * b:66 * b + 64],
                                     rhs=qs_t[1][:], start=False, stop=False)
                    nc.tensor.matmul(pat[:],
                                     lhsT=vsp_all[:, b * D:(b + 1) * D],
                                     rhs=w2b[:], start=False, stop=True)
                    nc.scalar.activation(
                        attn_sb[b // 2][(b % 2) * D:(b % 2 + 1) * D, :],
                        pat[:], ACTF.Copy)

                for p in range(4):
                    gt = wk.tile([128, S], F32, tag="gt", bufs=2)
                    nc.vector.tensor_mul(gt[:], pg[p][:], rrs_bc[:])
                    ot = wk.tile([128, S], F32, tag="ot", bufs=2)
                    nc.vector.tensor_add(ot[:], gt[:], attn_sb[p][:])
                    nc.sync.dma_start(out=out[2 * p], in_=ot[0:D, :])
                    nc.sync.dma_start(out=out[2 * p + 1], in_=ot[D:128, :])
    nc.compile()
    return nc


def _prep_inputs(query_input, source_input, adj, Wq_w, Wq_b, Wk_w, Wk_b,
                 Wv_w, Wv_b):
    bf16 = ml_dtypes.bfloat16
    fp8 = ml_dtypes.float8_e4m3fn
    xq_np = np.asarray(query_input, dtype=np.float32)
    xs_np = np.asarray(source_input, dtype=np.float32)
    adj_np = np.asarray(adj, dtype=np.float32)

    adjT = np.ascontiguousarray(adj_np.T)
    np.fill_diagonal(adjT, adjT.diagonal() + 1.0)
    adjT_f8 = adjT.astype(fp8)
    rrs_full = (0.25 / (adj_np.sum(axis=1) + 1.0)).astype(np.float32)

    wkv_np = np.concatenate([np.asarray(Wk_w, np.float32).T,
                             np.asarray(Wv_w, np.float32).T], axis=1)
    wkv_np = np.ascontiguousarray(wkv_np).astype(bf16).reshape(2, 128, 2 * HD)
    bkv_np = np.concatenate([np.asarray(Wk_b, np.float32),
                             np.asarray(Wv_b, np.float32)])
    bkv_np = bkv_np.astype(bf16).reshape(1, 2 * HD)
    wqt = np.ascontiguousarray(np.asarray(Wq_w, np.float32).T)
    wqt = wqt.astype(bf16).reshape(2, 128, HD)
    bqc = np.asarray(Wq_b, np.float32).reshape(2, 128, 1)

    i4o, ibcb = _indicators()
    in_maps = []
    for i in range(NCORES):
        sl = slice(i * S, (i + 1) * S)
        # [4096, S] -> [peer, jp, o, ki, n] -> [peer, jp, ki, o, n]
        a = adjT_f8[:, sl].reshape(8, 2, 2, 128, S)
        a = np.ascontiguousarray(a.transpose(0, 1, 3, 2, 4))
        in_maps.append({
            "xq": np.ascontiguousarray(xq_np[:, :, sl]).astype(bf16)
                  .reshape(B, 2, 128, S),
            "xs": np.ascontiguousarray(xs_np[:, :, sl]).astype(bf16)
                  .reshape(B, 2, 128, S),
            "adjt": a.reshape(16, 128, 2 * S),
            "wkv": wkv_np, "bkv": bkv_np, "wqt": wqt, "bqc": bqc,
            "i4o_in": i4o.astype(bf16),
            "ibcb_in": ibcb.astype(bf16),
            "orr_in": np.concatenate(
                [np.ones((1, S), np.float32),
                 rrs_full[sl].astype(np.float32).reshape(1, S)],
                axis=1).astype(bf16),
        })
    return in_maps


def kernel(**inputs):
    if "nc" not in _CACHE:
        _CACHE["nc"] = _build()
    nc = _CACHE["nc"]
    in_maps = _prep_inputs(**inputs)
    res = run_bass_kernel_spmd(nc, in_maps, list(range(NCORES)))
    full = np.empty((B, D, N), np.float32)
    for i in range(NCORES):
        full[:, :, i * S:(i + 1) * S] = res.results[i]["out"]
    return full
